# revision 71
# baseline (speedup 1.0000x reference)
"""Trainium2 Bass kernel for fused LN + QKV + partial-RoPE attention + out-proj.

Sharding: 8 cores = 4 batches x 2 head-groups (4 heads each).
Core c: batch = c % 4, heads = [4*(c//4) .. 4*(c//4)+3].
Each core returns a partial y^T [DIM, N] (f16); host sums the two
head-group partials per batch and adds b_out.

Design (173.8us; vs 186.5us prior / 265us original):
  - Dual bottleneck: ACT exp (~137us) and PE matmuls (~140us) are both
    near the wall; every change must keep both streaming.
  - exp in [128,1024] tiles (2 matmuls fill a 2-bank PSUM tile) to
    amortize ACT's fixed per-instruction access latency.
  - Softmax denominator: f16 pair-sum tree (L1 DVE, L2 Pool, L3 Pool)
    with the ENTIRE final tree column (s1_15/s2_3/s3b) plus the 2-link
    ones-matmul R chain deferred into the NEXT block at jb=FR_JB(3);
    by then the inputs are long ready so neither the exp stream nor the
    PE queue ever waits on denominator work. Deferring R to the block
    boundary alone (instead of jb3) costs ~3us.
  - AV accumulator evacuated by a DVE copy (after the tree adds) - not
    ACT - so the exp stream never pays for it; the last block evacuates
    on ACT (idle by then) to keep the DVE finish chain clean.
  - RoPE rotate-half via a second pair-swapped weights matmul for all
    heads (q, qs, k, ks, v_folded = 5 weight blocks/head; w_out is
    folded into V so AV directly yields y^T partials).
  - Startup (first exp ~15.3us): x loaded as f16 [128,NT,128] with the
    8 startup tiles in the first DMA; SP issues DMAs in dependency
    order (x0:8, identones, w-head0, cos/sin b0, x8:16, rest); LN raw
    sums for tiles 0-7 as fused 4-tile 3-D reduces on DVE, tiles 8-15
    on the startup-idle ACT via Identity/Square accum; istd via a short
    Newton chain on DVE; head-0 q/k/v in 512-col chunks ordered q-hf0,
    k-hf0, q-hf1, k-hf1 with jb0's scores+exp split into 512 halves so
    ACT starts as early as possible; ident-transpose warmup keeps the
    PE ramped through the LN wait.
  - Per-head QKV+rope work for head h+1 is interleaved into the emission
    stream of head h's attention in <=512-col steps.
  - y DMAs issue from the Pool queue (cheap dispatch, SP stays free).
  - PSUM: scores pool 3x[128,1024]f32 (6 banks, also serves QKV/xnT/R
    tiles) + single AV accumulator (2 banks). This is exactly full; a
    manual single-tile arena or wider exp groups do NOT fit/pay (tried:
    subtile-dep serialization, pair-WAR stalls).
"""

import numpy as np
import ml_dtypes
from contextlib import ExitStack

import concourse.bass as bass
import concourse.tile as tile
from concourse import bacc
from concourse import mybir
from concourse import bass_isa
from concourse.bass import ts
from concourse.bass_utils import run_bass_kernel_spmd

B, N, DIM = 4, 2048, 128
HEADS, HEAD = 8, 128
INNER = HEADS * HEAD
HPC = 4            # heads per core
NT = N // 128      # 16 token tiles
WQ = 1024          # attention q-block width
NQB = N // WQ      # 2 q-blocks per head
JT = 16            # j sub-blocks (128 each) per q-block
EPS = 1e-5
SCALE = HEAD ** -0.5

F32 = mybir.dt.float32
F16 = mybir.dt.float16
AF = mybir.ActivationFunctionType
ALU = mybir.AluOpType
AX = mybir.AxisListType

F16_NP = np.float16

_CACHE = {}
DEBUG_DUMPS = False
DRAIN = 0
DRAIN_FIRST = 0
ROPE_ADD_POOL = True
L2_POOL_COUNT = 3
N_WARM = 24
PENDING_JB = 5
FEED_H0 = 2
BOUNDARY_FEED = 0
S1_POOL_PAR = 2  # 2 = never on pool
PRE_SCORES = True
FR_JB = 3
FR_DEPTH4 = False
DRAIN_JB = 6
KDRAIN_JB = 4

# weight-pack layout: every head has 5 blocks (q, q_swap, k, k_swap,
# v_folded); rotate-half comes from a second pair-swapped weights matmul
# (GPSIMD cannot touch PSUM on real HW, so SBUF round-trips are avoided).
W_COLS = 5 * HPC


def _wslot(h, which):
    return 5 * h + {"q": 0, "qs": 1, "k": 2, "ks": 3, "v": 4}[which]


def _build_nc():
    nc = bacc.Bacc()
    x_d = nc.declare_dram_parameter("x", [128, NT, 128], F16, isOutput=False)
    wqkv_d = nc.declare_dram_parameter("wqkv", [128, W_COLS * 128], F16, isOutput=False)
    # packed [cos_b0 | sin_b0 | cos_b1 | sin_b1], each [128, WQ]
    cossin_d = nc.declare_dram_parameter("cossin", [128, 4 * WQ], F16, isOutput=False)
    identones_d = nc.declare_dram_parameter("identones", [128, 256], F16, isOutput=False)
    yt_d = nc.declare_dram_parameter("yt", [128, N], F16, isOutput=True)
    if DEBUG_DUMPS:
        dbg = {name: nc.declare_dram_parameter(f"dbg_{name}", [128, N], F16,
                                               isOutput=True)
               for name in ("xn", "xnT", "qh0", "kh0", "vh0", "e0", "r0")}

    with ExitStack() as ctx:
        tc = ctx.enter_context(tile.TileContext(nc))
        const = ctx.enter_context(tc.tile_pool(name="const", bufs=1))
        sq_p = ctx.enter_context(tc.tile_pool(name="sq", bufs=3))
        raw_p = ctx.enter_context(tc.tile_pool(name="raw", bufs=4))
        swp_p = ctx.enter_context(tc.tile_pool(name="swp", bufs=4))
        t12_p = ctx.enter_context(tc.tile_pool(name="t12", bufs=8))
        qh_p = ctx.enter_context(tc.tile_pool(name="qh", bufs=4))
        vh_p = ctx.enter_context(tc.tile_pool(name="vh", bufs=2))
        e_p = ctx.enter_context(tc.tile_pool(name="exps", bufs=10))
        s1_p = ctx.enter_context(tc.tile_pool(name="s1", bufs=8))
        s2_p = ctx.enter_context(tc.tile_pool(name="s2", bufs=6))
        s34_p = ctx.enter_context(tc.tile_pool(name="s34", bufs=4))
        rsb_p = ctx.enter_context(tc.tile_pool(name="rsb", bufs=2))
        rv_p = ctx.enter_context(tc.tile_pool(name="rv", bufs=3))
        osb_p = ctx.enter_context(tc.tile_pool(name="osb", bufs=3))
        on_p = ctx.enter_context(tc.tile_pool(name="onorm", bufs=6))
        y_p = ctx.enter_context(tc.tile_pool(name="y", bufs=4))
        ps_sc = ctx.enter_context(tc.tile_pool(name="ps_sc", bufs=3, space="PSUM"))
        ps_av = ctx.enter_context(tc.tile_pool(name="ps_av", bufs=1, space="PSUM"))

        # ---------------- input DMAs ----------------
        # SP queue, startup-critical order: the first exp needs (in chain
        # order) x[0:1024] -> LN -> xnT, head-0 q/k weights, cos/sin block 0.
        # The late halves (wqkv rest, cos/sin block 1) issue from the Pool
        # queue AFTER the xn ops so they can't steal early HWDGE slots.
        xt_all = const.tile([128, NT, 128], F16, tag="xt", name="xt_all")
        nc.sync.dma_start(out=xt_all[:, 0:8, :], in_=x_d[:, 0:8, :])
        identones_t = const.tile([128, 256], F16, tag="identones",
                                 name="identones_t")
        nc.sync.dma_start(out=identones_t, in_=identones_d[:, :])
        wqkv_t = const.tile([128, W_COLS * 128], F16, tag="wqkv", name="wqkv_t")
        nc.sync.dma_start(out=wqkv_t[:, 0:640], in_=wqkv_d[:, 0:640])
        nc.sync.dma_start(out=xt_all[:, 8:16, :], in_=x_d[:, 8:16, :])
        cossin_t = const.tile([128, 4 * WQ], F16, tag="cossin", name="cossin_t")
        nc.sync.dma_start(out=cossin_t[:, 0:2 * WQ], in_=cossin_d[:, 0:2 * WQ])
        nc.sync.dma_start(out=wqkv_t[:, 640:W_COLS * 128],
                          in_=wqkv_d[:, 640:W_COLS * 128])
        nc.sync.dma_start(out=cossin_t[:, 2 * WQ:4 * WQ],
                          in_=cossin_d[:, 2 * WQ:4 * WQ])
        ident_t = identones_t[:, 0:128]
        ones_t = identones_t[:, 128:256]

        def cosv(b):
            return cossin_t[:, b * 2 * WQ: b * 2 * WQ + WQ]

        def sinv(b):
            return cossin_t[:, b * 2 * WQ + WQ: (b + 1) * 2 * WQ]

        def W(h, which):
            return wqkv_t[:, ts(_wslot(h, which), 128)]

        def xtile(t):
            return xt_all[:, t, :]

        # ---------------- LayerNorm (per 8-tile half) ----------------
        st_sum = const.tile([128, NT], F32, tag="st_sum", name="st_sum")
        st_sq = const.tile([128, NT], F32, tag="st_sq", name="st_sq")
        mean = const.tile([128, NT], F32, tag="mean", name="mean")
        msq = const.tile([128, NT], F32, tag="msq", name="msq")
        lnv = const.tile([128, NT], F32, tag="lnv", name="lnv")
        istd = const.tile([128, NT], F32, tag="istd", name="istd")
        epsb = const.tile([128, 1], F32, tag="epsb", name="epsb")
        nc.vector.memset(epsb, EPS)
        xn = const.tile([128, N], F16, tag="xn", name="xn")
        xnT = const.tile([128, N], F16, tag="xnT", name="xnT")

        def ln_stats4(lo):
            """Raw sums for 4 token tiles in 3 wide DVE ops (3-D access
            pattern: reduce innermost d only)."""
            xg = xt_all[:, lo:lo + 4, :]
            nc.vector.tensor_reduce(
                out=st_sum[:, lo:lo + 4], in_=xg, axis=AX.X, op=ALU.add)
            sq4 = sq_p.tile([128, 4, 128], F16, tag="sq4", name=f"sq4_{lo}")
            nc.vector.tensor_mul(sq4, xg, xg)
            nc.vector.tensor_reduce(
                out=st_sq[:, lo:lo + 4], in_=sq4, axis=AX.X, op=ALU.add)

        def ln_stats_act(lo, hi):
            """Stats via the (startup-idle) ACT engine's free-axis
            accumulator; Identity/Square live in the Exp table set."""
            for t in range(lo, hi):
                scr = sq_p.tile([128, 128], F16, tag="ascr", name=f"as{t}")
                nc.scalar.activation(out=scr, in_=xtile(t), func=AF.Identity,
                                     accum_out=st_sum[:, t:t + 1])
                scr2 = sq_p.tile([128, 128], F16, tag="ascr", name=f"aq{t}")
                nc.scalar.activation(out=scr2, in_=xtile(t), func=AF.Square,
                                     accum_out=st_sq[:, t:t + 1])

        def ln_small(lo, hi):
            """mean/var/istd chain (small DVE ops) for a tile range."""
            sl = slice(lo, hi)
            nc.vector.tensor_scalar_mul(mean[:, sl], st_sum[:, sl], 1.0 / DIM)
            nc.vector.tensor_mul(msq[:, sl], mean[:, sl], mean[:, sl])
            nc.vector.scalar_tensor_tensor(
                out=lnv[:, sl], in0=st_sq[:, sl], scalar=1.0 / DIM,
                in1=msq[:, sl], op0=ALU.mult, op1=ALU.subtract)
            nc.vector.tensor_scalar_add(lnv[:, sl], lnv[:, sl], EPS)
            w = lnv[:, sl]
            nc.vector.reciprocal_approx_fast(out=w, in_=w)
            nc.vector.tensor_scalar(istd[:, sl], w, 0.5, 0.5, ALU.mult, ALU.add)
            tmp = sq_p.tile([128, NT], F32, tag="nt", name=f"nt{lo}")
            tr = tmp[:, sl]
            for _ in range(2):
                nc.vector.reciprocal_approx_fast(out=tr, in_=istd[:, sl])
                nc.vector.scalar_tensor_tensor(
                    out=tr, in0=tr, scalar=0.5, in1=w,
                    op0=ALU.mult, op1=ALU.mult)
                nc.vector.scalar_tensor_tensor(
                    out=istd[:, sl], in0=istd[:, sl], scalar=0.5, in1=tr,
                    op0=ALU.mult, op1=ALU.add)

        def ln_xn_steps(lo, hi, xnT_ps, xn_engine=None):
            eng = xn_engine or nc.gpsimd
            for t in range(lo, hi):
                eng.tensor_scalar(
                    xn[:, ts(t, 128)], xtile(t), mean[:, t:t + 1],
                    istd[:, t:t + 1], ALU.subtract, ALU.mult)
            yield 100
            for t in range(lo, hi):
                nc.tensor.transpose(
                    out=xnT_ps[:, ts(t % 8, 128)], in_=xn[:, ts(t, 128)],
                    identity=ident_t)
                if t == lo + 2:
                    yield 300
            base = (lo // 8) * WQ
            nc.vector.tensor_copy(
                xnT[:, base + (lo % 8) * 128: base + (hi % 8 or 8) * 128],
                xnT_ps[:, (lo % 8) * 128:(hi % 8 or 8) * 128])
            yield 250

        def ln_xn(lo, hi, xnT_ps, xn_engine=None):
            for _ in ln_xn_steps(lo, hi, xnT_ps, xn_engine):
                pass

        # keep the PE continuously busy through the LN wait so it reaches
        # full pstate before the first real matmuls (ramp: mid until 3us of
        # continuous execution). Warm matmuls use a memset tile so they are
        # not gated on any input DMA.
        warm = ps_sc.tile([128, 512], F16, tag="sc", name="warm")
        for i in range(N_WARM):
            nc.tensor.transpose(out=warm[:, 0:128], in_=ident_t,
                                identity=ident_t)

        xnT_ps0 = ps_sc.tile([128, WQ], F16, tag="sc", name="xnT_ps0")
        ln_stats4(0)
        ln_small(0, 4)
        ln_xn(0, 4, xnT_ps0, nc.vector)
        ln_stats4(4)
        ln_small(4, 8)
        ln_xn(4, 8, xnT_ps0, nc.vector)
        ln_stats_act(8, 12)      # ACT: idle until the first exp
        ln_stats_act(12, 16)
        qh_t, kh_t, vh_t = {}, {}, {}
        vh_pre = {}

        # ---------------- head-0 QKV (startup path, rot via matmul) ------
        def h0_qk_block_steps(which, wsw, dst, b):
            p_ps = ps_sc.tile([128, WQ], F32, tag="sc", name=f"h0p{which}{b}")
            pr_ps = ps_sc.tile([128, WQ], F32, tag="sc", name=f"h0r{which}{b}")
            for src_w in (which, wsw):
                dst_ps = p_ps if src_w == which else pr_ps
                for hf in range(WQ // 512):
                    nc.tensor.matmul(
                        out=dst_ps[:, ts(hf, 512)], lhsT=W(0, src_w),
                        rhs=xnT[:, b * WQ + hf * 512: b * WQ + (hf + 1) * 512],
                        start=True, stop=True, skip_group_check=True)
                    yield 430
            t1 = t12_p.tile([128, WQ], F16, tag="t12", name=f"h0t1{which}{b}")
            nc.vector.tensor_mul(t1, p_ps, cosv(b))
            yield 600
            t2 = t12_p.tile([128, WQ], F16, tag="t12", name=f"h0t2{which}{b}")
            nc.vector.tensor_mul(t2, pr_ps, sinv(b))
            yield 600
            nc.vector.tensor_add(dst[:, ts(b, WQ)], t1, t2)
            yield 300

        def h0_qk_first(qh0, kh0):
            """q+k block-0 in 512-col chunks (one PSUM pair live at a time).
            k hf1 (j-tiles 4-7) is deferred to the feed: jb0-3 only need
            k[0:512], and every exp needs the full q block."""
            for hf, which, wsw, dst in ((0, "q", "qs", qh0), (0, "k", "ks", kh0),
                                        (1, "q", "qs", qh0), (1, "k", "ks", kh0)):
                sl = slice(hf * 512, (hf + 1) * 512)
                p_ps = ps_sc.tile([128, 512], F32, tag="sc",
                                  name=f"h0p{which}{hf}")
                pr_ps = ps_sc.tile([128, 512], F32, tag="sc",
                                   name=f"h0r{which}{hf}")
                nc.tensor.matmul(out=p_ps, lhsT=W(0, which), rhs=xnT[:, sl],
                                 start=True, stop=True, skip_group_check=True)
                nc.tensor.matmul(out=pr_ps, lhsT=W(0, wsw), rhs=xnT[:, sl],
                                 start=True, stop=True, skip_group_check=True)
                t1 = t12_p.tile([128, 512], F16, tag="t12h0",
                                name=f"f1{which}{hf}")
                nc.vector.tensor_mul(t1, p_ps, cossin_t[:, sl])
                t2 = t12_p.tile([128, 512], F16, tag="t12h0",
                                name=f"f2{which}{hf}")
                nc.vector.tensor_mul(
                    t2, pr_ps, cossin_t[:, WQ + hf * 512: WQ + (hf + 1) * 512])
                nc.gpsimd.tensor_add(dst[:, sl], t1, t2)

        def h0_k1_steps(kh0):
            sl = slice(512, 1024)
            p_ps = ps_sc.tile([128, 512], F32, tag="sc", name="h0pk1")
            pr_ps = ps_sc.tile([128, 512], F32, tag="sc", name="h0rk1")
            nc.tensor.matmul(out=p_ps, lhsT=W(0, "k"), rhs=xnT[:, sl],
                             start=True, stop=True, skip_group_check=True)
            nc.tensor.matmul(out=pr_ps, lhsT=W(0, "ks"), rhs=xnT[:, sl],
                             start=True, stop=True, skip_group_check=True)
            yield 300
            t1 = t12_p.tile([128, 512], F16, tag="t12h0", name="f1k1d")
            nc.vector.tensor_mul(t1, p_ps, cossin_t[:, sl])
            yield 500
            t2 = t12_p.tile([128, 512], F16, tag="t12h0", name="f2k1d")
            nc.vector.tensor_mul(
                t2, pr_ps, cossin_t[:, WQ + 512: WQ + 1024])
            yield 500
            nc.vector.tensor_add(kh0[:, sl], t1, t2)
            yield 300

        def v_block_steps(h, vh, b, split=False):
            v_ps = ps_sc.tile([128, WQ], F32, tag="sc", name=f"v{h}{b}")
            for c in range(WQ // 128):
                nc.tensor.matmul(
                    out=v_ps[:, ts(c, 128)],
                    lhsT=xnT[:, b * WQ + c * 128: b * WQ + (c + 1) * 128],
                    rhs=W(h, "v"), start=True, stop=True,
                    skip_group_check=True)
                if c == WQ // 256:
                    yield 215
            if split:
                # early AV j-tiles only need the first v columns
                nc.vector.tensor_copy(vh[:, b * WQ: b * WQ + 128],
                                      v_ps[:, 0:128])
                nc.vector.tensor_copy(vh[:, b * WQ + 128: b * WQ + 512],
                                      v_ps[:, 128:512])
                nc.vector.tensor_copy(vh[:, b * WQ + 512: (b + 1) * WQ],
                                      v_ps[:, 512:WQ])
            else:
                nc.vector.tensor_copy(vh[:, ts(b, WQ)], v_ps)
            yield 100

        def prologue_steps():
            """Head-0 setup. Block-0 q/k/v are inline (startup path); the
            rest (LN tail for tiles 8-15, block-1 q/k/v) is fed into head-0's
            first attention block in small steps."""
            qh0 = qh_p.tile([128, N], F16, tag="qh", name="qh0")
            kh0 = qh_p.tile([128, N], F16, tag="qh", name="kh0")
            vh0 = vh_p.tile([128, N], F16, tag="vh", name="vh0")
            qh_t[0], kh_t[0], vh_t[0] = qh0, kh0, vh0
            h0_qk_first(qh0, kh0)
            for _ in v_block_steps(0, vh0, 0, split=True):
                pass
            yield "prologue-ready"
            ln_small(8, 16)
            yield 200
            xnT_ps1 = ps_sc.tile([128, WQ], F16, tag="sc", name="xnT_ps1")
            for st in ln_xn_steps(8, 12, xnT_ps1):
                yield st
            for st in ln_xn_steps(12, 16, xnT_ps1):
                yield st
            for st in qk_rope_block(0, "k", "ks", kh0, 1, nc.vector):
                yield st
            yield "b1-k-done"
            for st in v_block_steps(0, vh0, 1):
                yield st
            vh1 = vh_p.tile([128, N], F16, tag="vh", name="vh1")
            vh_pre[1] = vh1
            yield 200
            v_block_act(1, vh1, 0)
            yield 430
            v_block_act(1, vh1, 1)
            yield "b1-kv-done"
            for st in qk_rope_block(0, "q", "qs", qh0, 1, nc.vector):
                yield st
            yield "pgen-done"

        # ---------------- per-head QKV (rotate-half via matmul) ----------
        def qk_rope_block(h, which, wsw, dst, b, add_engine):
            """p/pr projections for one [WQ] block + rope: DVE multiplies
            evacuate PSUM, the all-SBUF f16 add runs on `add_engine`."""
            p_ps = ps_sc.tile([128, WQ], F32, tag="sc", name=f"p{h}{which}{b}")
            pr_ps = ps_sc.tile([128, WQ], F32, tag="sc", name=f"r{h}{which}{b}")
            for dst_ps, wname in ((p_ps, which), (pr_ps, wsw)):
                for hf in range(WQ // 512):
                    nc.tensor.matmul(
                        out=dst_ps[:, ts(hf, 512)], lhsT=W(h, wname),
                        rhs=xnT[:, b * WQ + hf * 512: b * WQ + (hf + 1) * 512],
                        start=True, stop=True, skip_group_check=True)
                yield 430
            sl = ts(b, WQ)
            t1 = t12_p.tile([128, WQ], F16, tag="t12", name=f"t1{h}{which}{b}")
            nc.vector.tensor_mul(t1, p_ps, cosv(b))
            yield 600
            t2 = t12_p.tile([128, WQ], F16, tag="t12", name=f"t2{h}{which}{b}")
            nc.vector.tensor_mul(t2, pr_ps, sinv(b))
            yield 600
            add_engine.tensor_add(dst[:, sl], t1, t2)
            yield 300

        def v_block_act(h, vh, b):
            """v projection with ACT-side PSUM evacuation (for windows where
            ACT is otherwise idle, i.e. before the exp stream starts)."""
            v_ps = ps_sc.tile([128, WQ], F32, tag="sc", name=f"v{h}{b}")
            for c in range(WQ // 128):
                nc.tensor.matmul(
                    out=v_ps[:, ts(c, 128)],
                    lhsT=xnT[:, b * WQ + c * 128: b * WQ + (c + 1) * 128],
                    rhs=W(h, "v"), start=True, stop=True,
                    skip_group_check=True)
            nc.scalar.copy(out=vh[:, ts(b, WQ)], in_=v_ps)

        def qkv_steps(h):
            qh = qh_p.tile([128, N], F16, tag="qh", name=f"qh{h}")
            kh = qh_p.tile([128, N], F16, tag="qh", name=f"kh{h}")
            for dst, which, wsw in ((qh, "q", "qs"), (kh, "k", "ks")):
                for b in range(NQB):
                    for st in qk_rope_block(h, which, wsw, dst, b,
                                            nc.gpsimd if ROPE_ADD_POOL else nc.vector):
                        yield st
            qh_t[h], kh_t[h] = qh, kh
            if h == 1:
                vh_t[h] = vh_pre[h]     # computed in the prologue
                return
            vh = vh_p.tile([128, N], F16, tag="vh", name=f"vh{h}")
            for b in range(NQB):
                for st in v_block_steps(h, vh, b):
                    yield st
            vh_t[h] = vh

        # ---------------- attention ----------------
        onb_t = {}
        pair_t = {}
        pre_scores = {}

        def attention_block(h, qb, feed, pending, drain_feed, finish_prev):
            qh, kh, vh = qh_t[h], kh_t[h], vh_t[h]
            qsl = qh[:, ts(qb, WQ)]
            last = (h == HPC - 1 and qb == NQB - 1)

            def scores(jb):
                s_ps = ps_sc.tile([128, WQ], F32, tag="sc", name=f"s{h}{qb}{jb}")
                for hf in range(WQ // 512):
                    nc.tensor.matmul(
                        out=s_ps[:, ts(hf, 512)], lhsT=kh[:, ts(jb, 128)],
                        rhs=qsl[:, ts(hf, 512)], start=True, stop=True,
                        skip_group_check=True)
                return s_ps

            o_acc = ps_av.tile([128, WQ], F32, tag="av", name=f"oacc{h}{qb}")
            if h == 0 and qb == 1:
                drain_feed("pgen-done")   # qh0 block-1 emitted before use
            if h == 0 and qb == 0:
                s0a = ps_sc.tile([128, 512], F32, tag="sc", name="s00a")
                nc.tensor.matmul(out=s0a, lhsT=kh[:, 0:128], rhs=qsl[:, 0:512],
                                 start=True, stop=True, skip_group_check=True)
                s0b = ps_sc.tile([128, 512], F32, tag="sc", name="s00b")
                nc.tensor.matmul(out=s0b, lhsT=kh[:, 0:128],
                                 rhs=qsl[:, 512:1024],
                                 start=True, stop=True, skip_group_check=True)
                s_tiles = {0: (s0a, s0b), 1: scores(1)}
            elif (h, qb) in pre_scores:
                s_tiles = pre_scores.pop((h, qb))
            else:
                s_tiles = {0: scores(0), 1: scores(1)}
            # the previous block's R chain runs AFTER this block's first two
            # scores so the exp stream never waits on the denominator matmuls
            if finish_prev is not None and FR_JB < 0:
                finish_prev()
            if not (h == 0 and qb == 0):
                for _ in range(BOUNDARY_FEED):
                    feed()
            es, s1s, s2s, s3s = [], [], [], []
            osb = None
            for jb in range(JT):
                if h == 0 and qb == 0 and jb == KDRAIN_JB:
                    drain_feed("b1-k-done")
                if h == 0 and qb == 0 and jb == DRAIN_JB:
                    # all remaining head-0 prologue (kh0/vh0/qh0 block-1)
                    # must be emitted before the instructions that read it
                    # (emission order is program order; a later write would
                    # read as WAR)
                    drain_feed("pgen-done")
                if jb + 2 < JT and jb + 2 not in s_tiles:
                    s_tiles[jb + 2] = scores(jb + 2)
                e = e_p.tile([128, WQ], F16, tag="expT", name=f"e{h}{qb}{jb}")
                s_in = s_tiles.pop(jb)
                if isinstance(s_in, tuple):
                    nc.scalar.activation(out=e[:, 0:512], in_=s_in[0],
                                         func=AF.Exp, scale=SCALE)
                    nc.scalar.activation(out=e[:, 512:1024], in_=s_in[1],
                                         func=AF.Exp, scale=SCALE)
                else:
                    nc.scalar.activation(out=e, in_=s_in, func=AF.Exp,
                                         scale=SCALE)
                es.append(e)
                if DEBUG_DUMPS and DEBUG_DUMPS != 2 and h == 0 and qb == 0 and jb < 2:
                    nc.sync.dma_start(out=dbg["e0"][:, ts(jb, WQ)], in_=e)
                if DEBUG_DUMPS == 2 and h == 1 and qb == 0 and jb in (5, 12):
                    nc.sync.dma_start(out=dbg["e0"][:, ts(int(jb == 12), WQ)], in_=e)
                if jb == FR_JB and finish_prev is not None:
                    finish_prev()
                if jb == PENDING_JB and pending is not None:
                    pending()
                feed()
                if h == 0 and qb == 0:
                    for _ in range(FEED_H0 - 1):
                        feed()
                if jb == JT - 1 and qb == 0 and PRE_SCORES:
                    # the PE otherwise idles on exp(15) before the final AV
                    # group; the sibling q-block's kh/qh are already live
                    qsl1 = qh[:, ts(1, WQ)]
                    pre = {}
                    for j2 in range(2):
                        sp = ps_sc.tile([128, WQ], F32, tag="sc",
                                        name=f"ps{h}1{j2}")
                        for hf in range(WQ // 512):
                            nc.tensor.matmul(
                                out=sp[:, ts(hf, 512)], lhsT=kh[:, ts(j2, 128)],
                                rhs=qsl1[:, ts(hf, 512)], start=True, stop=True,
                                skip_group_check=True)
                        pre[j2] = sp
                    pre_scores[(h, 1)] = pre
                for hf in range(WQ // 512):
                    nc.tensor.matmul(
                        out=o_acc[:, ts(hf, 512)], lhsT=vh[:, ts(jb, 128)],
                        rhs=e[:, ts(hf, 512)], start=(jb == 0),
                        stop=(jb == JT - 1), skip_group_check=True)
                if jb % 2 == 1:
                    if jb == JT - 1 and not last:
                        pass      # whole final tree column deferred to finish_R
                    else:
                        s1 = s1_p.tile([128, WQ], F16, tag="s1",
                                       name=f"s1_{h}{qb}{jb}")
                        if last and jb == JT - 1:
                            # final links in 512 halves: the R chain for half 0
                            # starts one link earlier after the last exp
                            for hf in range(WQ // 512):
                                nc.vector.tensor_add(s1[:, ts(hf, 512)],
                                                     es[-2][:, ts(hf, 512)],
                                                     es[-1][:, ts(hf, 512)])
                        else:
                            nc.vector.tensor_add(s1, es[-2], es[-1])
                        s1s.append(s1)
                if jb % 4 == 3:
                    if jb == JT - 1 and not last:
                        pass      # deferred to finish_R
                    else:
                        s2 = s2_p.tile([128, WQ], F16, tag="s2",
                                       name=f"s2_{h}{qb}{jb}")
                        eng = (nc.gpsimd if len(s2s) < L2_POOL_COUNT and
                               not last else nc.vector)
                        if last and jb == JT - 1:
                            for hf in range(WQ // 512):
                                eng.tensor_add(s2[:, ts(hf, 512)],
                                               s1s[-2][:, ts(hf, 512)],
                                               s1s[-1][:, ts(hf, 512)])
                        else:
                            eng.tensor_add(s2, s1s[-2], s1s[-1])
                        s2s.append(s2)
                    if not last and jb == 7:
                        # L3 first half on Pool (plenty of time mid-block)
                        s3 = s34_p.tile([128, WQ], F16, tag="s34",
                                        name=f"s3_{h}{qb}{jb}")
                        nc.gpsimd.tensor_add(s3, s2s[-2], s2s[-1])
                        s3s.append(s3)
                if jb == JT - 1:
                    # AV accumulator evacuation frees the single PSUM slot.
                    # Emitted AFTER the tree adds so the R chain isn't stuck
                    # behind the 1.2us copy on DVE; the last block uses ACT
                    # (idle by then) to keep the DVE finish chain clean.
                    osb = osb_p.tile([128, WQ], F32, tag="osb",
                                     name=f"osb{h}{qb}")
                    if last:
                        nc.scalar.copy(out=osb, in_=o_acc)
                    else:
                        nc.vector.tensor_copy(osb, o_acc)
            rinv = rv_p.tile([128, WQ], F32, tag="rinv", name=f"rinv{h}{qb}")
            if last:
                # R inline, per-half chain + reciprocal so the finish pipeline
                # (norm/pair/y/dma) starts on half 0 while half 1 reduces
                R_ps = ps_sc.tile([128, WQ], F32, tag="sc", name=f"R{h}{qb}")
                for hf in range(WQ // 512):
                    for u, s2 in enumerate(s2s):
                        nc.tensor.matmul(
                            out=R_ps[:, ts(hf, 512)], lhsT=ones_t,
                            rhs=s2[:, ts(hf, 512)], start=(u == 0),
                            stop=(u == len(s2s) - 1), skip_group_check=True)
                    nc.vector.reciprocal_approx_fast(
                        out=rinv[:, ts(hf, 512)], in_=R_ps[:, ts(hf, 512)])

            def finish_R():
                # deferred into the next block (FR_JB): the final tree column
                # (s1_15, s2_3, s3b) runs here too, far from the boundary
                s1t = s1_p.tile([128, WQ], F16, tag="s1", name=f"s1t{h}{qb}")
                nc.vector.tensor_add(s1t, es[-2], es[-1])
                s2t = s2_p.tile([128, WQ], F16, tag="s2", name=f"s2t{h}{qb}")
                nc.vector.tensor_add(s2t, s1s[-1], s1t)
                s3t = s34_p.tile([128, WQ], F16, tag="s34", name=f"s3t{h}{qb}")
                nc.vector.tensor_add(s3t, s2s[-1], s2t)
                R_ps = ps_sc.tile([128, WQ], F32, tag="sc", name=f"R{h}{qb}")
                for u, s3 in enumerate((s3s[0], s3t)):
                    for hf in range(WQ // 512):
                        nc.tensor.matmul(
                            out=R_ps[:, ts(hf, 512)], lhsT=ones_t,
                            rhs=s3[:, ts(hf, 512)], start=(u == 0),
                            stop=(u == 1), skip_group_check=True)
                nc.vector.reciprocal_approx_fast(out=rinv, in_=R_ps)

            def tail():
                onb = on_p.tile([128, WQ], F16, tag="onb", name=f"onb{h}{qb}")
                nc.gpsimd.tensor_mul(onb, osb, rinv)
                onb_t[(h, qb)] = onb
                if h == 1:
                    pr = y_p.tile([128, WQ], F16, tag="pair", name=f"pr1{qb}")
                    nc.gpsimd.tensor_add(pr, onb_t[(0, qb)], onb_t[(1, qb)])
                    pair_t[(0, qb)] = pr
                if h == 2:
                    # pre-sum heads 0..2 so the final y is one add per chunk
                    pr = y_p.tile([128, WQ], F16, tag="pair", name=f"pre3{qb}")
                    nc.gpsimd.tensor_add(pr, pair_t[(0, qb)], onb_t[(2, qb)])
                    pair_t[(2, qb)] = pr
                if h == 3:
                    emit_y(qb, 1)

            if last:
                # low-latency all-DVE finish, fully chunked per 512
                onb = on_p.tile([128, WQ], F16, tag="onb", name=f"onb{h}{qb}")
                onb_t[(h, qb)] = onb
                y_sb = y_p.tile([128, WQ], F16, tag="ysb", name=f"ysb{qb}")
                for hf in range(WQ // 512):
                    sl = slice(hf * 512, (hf + 1) * 512)
                    nc.vector.tensor_mul(onb[:, sl], osb[:, sl], rinv[:, sl])
                    nc.vector.tensor_add(y_sb[:, sl], pair_t[(2, qb)][:, sl],
                                         onb[:, sl])
                    nc.gpsimd.dma_start(out=yt_d[:, qb * WQ + hf * 512:
                                                 qb * WQ + (hf + 1) * 512],
                                        in_=y_sb[:, sl])
                return None, None
            return finish_R, tail

        def emit_y(qb, chunks):
            y_sb = y_p.tile([128, WQ], F16, tag="ysb", name=f"ysb{qb}")
            for hf in range(chunks):
                w = WQ // chunks
                sl = slice(hf * w, (hf + 1) * w)
                nc.vector.tensor_add(y_sb[:, sl], pair_t[(2, qb)][:, sl],
                                     onb_t[(3, qb)][:, sl])
                nc.gpsimd.dma_start(out=yt_d[:, qb * WQ + hf * w:
                                             qb * WQ + (hf + 1) * w],
                                    in_=y_sb[:, sl])

        def run_head(h, feed, pending, drain_feed, finish_prev):
            for qb in range(NQB):
                finish_prev, pending = attention_block(
                    h, qb, feed, pending, drain_feed, finish_prev)
                feed_gate[0] = None   # qb0 of head 0 feeds the prologue only
            return finish_prev, pending

        # prologue: run until head-0 (qb0) inputs exist
        pgen = prologue_steps()
        next(pgen)
        if DEBUG_DUMPS == 2:
            pass  # keep interleaved schedule; dumps added at end
        elif DEBUG_DUMPS:
            for _ in pgen:
                pass
            nc.sync.dma_start(out=dbg["xn"][:, :], in_=xn)
            nc.sync.dma_start(out=dbg["xnT"][:, :], in_=xnT)
            nc.sync.dma_start(out=dbg["qh0"][:, :], in_=qh_t[0])
            nc.sync.dma_start(out=dbg["kh0"][:, :], in_=kh_t[0])
            nc.sync.dma_start(out=dbg["vh0"][:, :], in_=vh_t[0])

        pending = None
        finishR = None
        feed_gate = [None]   # when set, only this generator may be pulled
        for h in range(HPC):
            gens = []
            if h == 0:
                gens.append(pgen)
                feed_gate[0] = pgen
            if h + 1 < HPC:
                gens.append(qkv_steps(h + 1))
            if h == 0 and DRAIN_FIRST:
                for _ in range(DRAIN_FIRST):
                    try:
                        next(gens[0])
                    except StopIteration:
                        break
            if DRAIN == 2 or (DRAIN == 1 and h == 0):
                while gens:
                    try:
                        next(gens[0])
                    except StopIteration:
                        gens.pop(0)

            def feed(gs=gens):
                while gs:
                    try:
                        next(gs[0])
                        return
                    except StopIteration:
                        gs.pop(0)

            def drain_feed(marker=None, gs=gens):
                while gs:
                    try:
                        st = next(gs[0])
                        if marker is not None and st == marker:
                            return
                    except StopIteration:
                        gs.pop(0)

            finishR, pending = run_head(h, feed, pending, drain_feed, finishR)
            while gens:
                try:
                    next(gens[0])
                except StopIteration:
                    gens.pop(0)
        if pending is not None:
            pending()
        if DEBUG_DUMPS == 2:
            nc.sync.dma_start(out=dbg["xnT"][:, 0:WQ], in_=onb_t[(1, 0)])
            nc.sync.dma_start(out=dbg["xn"][:, :], in_=qh_t[1])
            nc.sync.dma_start(out=dbg["xnT"][:, :], in_=kh_t[1])
            nc.sync.dma_start(out=dbg["qh0"][:, :], in_=vh_t[1])
            nc.sync.dma_start(out=dbg["kh0"][:, :], in_=qh_t[2])
            nc.sync.dma_start(out=dbg["vh0"][:, :], in_=vh_t[2])
            nc.sync.dma_start(out=dbg["e0"][:, :], in_=qh_t[3])
            nc.sync.dma_start(out=dbg["r0"][:, :], in_=vh_t[3])

    nc.finalize()
    return nc


def _make_runner(nc, n_cores=8):
    """Cached jitted multi-core executor (mirrors bass2jax.run_bass_via_pjrt,
    minus output-donation so it can be called repeatedly for timing)."""
    import jax
    import jax.numpy as jnp
    from jax.sharding import Mesh, PartitionSpec
    from jax.experimental.shard_map import shard_map
    from concourse import bass2jax, mybir as mb
    bass2jax.install_neuronx_cc_hook()

    partition_name = nc.partition_id_tensor.name if nc.partition_id_tensor else None
    in_names, out_names, out_avals, zero_outs = [], [], [], []
    for alloc in nc.m.functions[0].allocations:
        if not isinstance(alloc, mb.MemoryLocationSet):
            continue
        name = alloc.memorylocations[0].name
        if alloc.kind == "ExternalInput":
            if name != partition_name:
                in_names.append(name)
        elif alloc.kind == "ExternalOutput":
            out_names.append(name)
            shape = tuple(alloc.tensor_shape)
            dtype = mb.dt.np(alloc.dtype)
            out_avals.append(jax.core.ShapedArray(shape, dtype))
            zero_outs.append(np.zeros(shape, dtype))
    n_params = len(in_names)
    all_in_names = list(in_names) + list(out_names)
    if partition_name is not None:
        all_in_names.append(partition_name)

    def _body(*args):
        operands = list(args)
        if partition_name is not None:
            operands.append(bass2jax.partition_id_tensor())
        outs = bass2jax._bass_exec_p.bind(
            *operands,
            out_avals=tuple(out_avals),
            in_names=tuple(all_in_names),
            out_names=tuple(out_names),
            lowering_input_output_aliases=(),
            sim_require_finite=True,
            sim_require_nnan=True,
            nc=nc,
        )
        return tuple(outs)

    devices = jax.devices()[:n_cores]
    mesh = Mesh(np.asarray(devices), ("core",))
    in_specs = (PartitionSpec("core"),) * (n_params + len(out_names))
    out_specs = (PartitionSpec("core"),) * len(out_names)
    donate = tuple(range(n_params, n_params + len(out_names)))
    sharded = jax.jit(shard_map(_body, mesh=mesh, in_specs=in_specs,
                                out_specs=out_specs, check_rep=False),
                      donate_argnums=donate, keep_unused=True)

    def run(in_maps):
        concat_in = [np.concatenate([np.asarray(in_maps[c][k]) for c in range(n_cores)], axis=0)
                     for k in in_names]
        concat_zero = [np.concatenate([z] * n_cores, axis=0) for z in zero_outs]
        outs = sharded(*concat_in, *concat_zero)
        outs = [np.asarray(o) for o in outs]
        res = []
        for c in range(n_cores):
            d = {}
            for i, name in enumerate(out_names):
                per = outs[i].shape[0] // n_cores
                d[name] = outs[i][c * per:(c + 1) * per]
            res.append(d)
        return res, sharded, (in_names, zero_outs)

    return run


def _rope_tables():
    """cos/sin tables in [d, n] layout; token N-1 unrotated; sin sign-folded."""
    inv_freq = 1.0 / (10000.0 ** (np.arange(0, HEAD, 2, dtype=np.float64) / HEAD))
    pos = np.arange(N, dtype=np.float64)
    ang = pos[None, :] * np.repeat(inv_freq, 2)[:, None]        # [d, n]
    cos_t = np.cos(ang)
    sin_t = np.sin(ang)
    sign = np.where(np.arange(HEAD) % 2 == 0, -1.0, 1.0)[:, None]
    sin_t = sin_t * sign
    cos_t[:, N - 1] = 1.0
    sin_t[:, N - 1] = 0.0
    return cos_t.astype(F16_NP), sin_t.astype(F16_NP)


def _prep_core_inputs(x, ln_gamma, ln_beta, w_qkv, w_out):
    """Build the 8 per-core input maps (host-side layout/packing)."""
    cos_t, sin_t = _rope_tables()
    ident = np.eye(128, dtype=np.float32)
    swap = np.arange(HEAD) ^ 1

    # packed [cos_b0 | sin_b0 | cos_b1 | sin_b1]
    cossin = np.concatenate(
        [cos_t[:, 0:WQ], sin_t[:, 0:WQ], cos_t[:, WQ:N], sin_t[:, WQ:N]],
        axis=1).astype(F16_NP)
    identones = np.concatenate(
        [ident.astype(F16_NP), np.ones((128, 128), dtype=F16_NP)], axis=1)

    in_maps = []
    for c in range(8):
        b = c % 4
        g = c // 4
        wq_blocks = []
        for i in range(HPC):
            h = g * HPC + i
            Wq = w_qkv[h * HEAD:(h + 1) * HEAD, :] * ln_gamma[None, :]
            Wk = w_qkv[INNER + h * HEAD:INNER + (h + 1) * HEAD, :] * ln_gamma[None, :]
            Wv = w_qkv[2 * INNER + h * HEAD:2 * INNER + (h + 1) * HEAD, :] * ln_gamma[None, :]
            Wo = w_out[:, h * HEAD:(h + 1) * HEAD]
            Wvo = Wo @ Wv                                        # fold out-proj into V
            wq_blocks += [Wq.T, Wq[swap, :].T, Wk.T, Wk[swap, :].T, Wvo.T]
        wqkv_packed = np.concatenate(wq_blocks, axis=1)          # [128, W_COLS*128]
        # pre-tile x to [128, NT, 128]: [p, t, d] holds x[b, t*128+p, d]
        xb = np.ascontiguousarray(
            x[b].reshape(NT, 128, DIM).transpose(1, 0, 2),
            dtype=F16_NP)
        in_maps.append({
            "x": xb,
            "wqkv": wqkv_packed.astype(F16_NP),
            "cossin": cossin,
            "identones": identones,
        })
    return in_maps


def kernel(x, ln_gamma, ln_beta, w_qkv, w_out, b_out):
    x = np.asarray(x, dtype=np.float32)
    ln_gamma = np.asarray(ln_gamma, dtype=np.float32)
    ln_beta = np.asarray(ln_beta, dtype=np.float32)
    w_qkv = np.asarray(w_qkv, dtype=np.float32)
    w_out = np.asarray(w_out, dtype=np.float32)
    b_out = np.asarray(b_out, dtype=np.float32)
    assert np.allclose(ln_beta, 0.0), "beta folding not implemented"

    if "nc" not in _CACHE:
        _CACHE["nc"] = _build_nc()
    nc = _CACHE["nc"]

    in_maps = _prep_core_inputs(x, ln_gamma, ln_beta, w_qkv, w_out)
    _CACHE["last_in_maps"] = in_maps
    res = run_bass_kernel_spmd(nc, in_maps, list(range(8)))
    results = res.results

    out = np.empty((B, N, DIM), dtype=np.float32)
    for b in range(B):
        y0 = np.asarray(results[b]["yt"], dtype=np.float32)
        y1 = np.asarray(results[b + 4]["yt"], dtype=np.float32)
        out[b] = (y0 + y1).T + b_out[None, :]
    return out



# revision 77
# speedup vs baseline: 1.0030x; 1.0030x over previous
"""Trainium2 Bass kernel for fused LN + QKV + partial-RoPE attention + out-proj.

Sharding: 8 cores = 4 batches x 2 head-groups (4 heads each).
Core c: batch = c % 4, heads = [4*(c//4) .. 4*(c//4)+3].
Each core returns a partial y^T [DIM, N] (f16); host sums the two
head-group partials per batch and adds b_out.

Design (173.8us; vs 186.5us prior / 265us original):
  - Dual bottleneck: ACT exp (~137us) and PE matmuls (~140us) are both
    near the wall; every change must keep both streaming.
  - exp in [128,1024] tiles (2 matmuls fill a 2-bank PSUM tile) to
    amortize ACT's fixed per-instruction access latency.
  - Softmax denominator: f16 pair-sum tree (L1 DVE, L2 Pool, L3 Pool)
    with the ENTIRE final tree column (s1_15/s2_3/s3b) plus the 2-link
    ones-matmul R chain deferred into the NEXT block at jb=FR_JB(3);
    by then the inputs are long ready so neither the exp stream nor the
    PE queue ever waits on denominator work. Deferring R to the block
    boundary alone (instead of jb3) costs ~3us.
  - AV accumulator evacuated by a DVE copy (after the tree adds) - not
    ACT - so the exp stream never pays for it; the last block evacuates
    on ACT (idle by then) to keep the DVE finish chain clean.
  - RoPE rotate-half via a second pair-swapped weights matmul for all
    heads (q, qs, k, ks, v_folded = 5 weight blocks/head; w_out is
    folded into V so AV directly yields y^T partials).
  - Startup (first exp ~15.3us): x loaded as f16 [128,NT,128] with the
    8 startup tiles in the first DMA; SP issues DMAs in dependency
    order (x0:8, identones, w-head0, cos/sin b0, x8:16, rest); LN raw
    sums for tiles 0-7 as fused 4-tile 3-D reduces on DVE, tiles 8-15
    on the startup-idle ACT via Identity/Square accum; istd via a short
    Newton chain on DVE; head-0 q/k/v in 512-col chunks ordered q-hf0,
    k-hf0, q-hf1, k-hf1 with jb0's scores+exp split into 512 halves so
    ACT starts as early as possible; ident-transpose warmup keeps the
    PE ramped through the LN wait.
  - Per-head QKV+rope work for head h+1 is interleaved into the emission
    stream of head h's attention in <=512-col steps.
  - y DMAs issue from the Pool queue (cheap dispatch, SP stays free).
  - PSUM: scores pool 3x[128,1024]f32 (6 banks, also serves QKV/xnT/R
    tiles) + single AV accumulator (2 banks). This is exactly full; a
    manual single-tile arena or wider exp groups do NOT fit/pay (tried:
    subtile-dep serialization, pair-WAR stalls).
"""

import numpy as np
import ml_dtypes
from contextlib import ExitStack

import concourse.bass as bass
import concourse.tile as tile
from concourse import bacc
from concourse import mybir
from concourse import bass_isa
from concourse.bass import ts
from concourse.bass_utils import run_bass_kernel_spmd

B, N, DIM = 4, 2048, 128
HEADS, HEAD = 8, 128
INNER = HEADS * HEAD
HPC = 4            # heads per core
NT = N // 128      # 16 token tiles
WQ = 1024          # attention q-block width
NQB = N // WQ      # 2 q-blocks per head
JT = 16            # j sub-blocks (128 each) per q-block
EPS = 1e-5
SCALE = HEAD ** -0.5

F32 = mybir.dt.float32
F16 = mybir.dt.float16
AF = mybir.ActivationFunctionType
ALU = mybir.AluOpType
AX = mybir.AxisListType

F16_NP = np.float16

_CACHE = {}
DEBUG_DUMPS = False
DRAIN = 0
DRAIN_FIRST = 0
ROPE_ADD_POOL = True
L2_POOL_COUNT = 3
N_WARM = 24
PENDING_JB = 5
FEED_H0 = 2
BOUNDARY_FEED = 0
S1_POOL_PAR = 2  # 2 = never on pool
PRE_SCORES = True
FR_JB = 3
FR_DEPTH4 = False
DRAIN_JB = 6
KDRAIN_JB = 4
EMITY_SP = False
TAIL_CHUNKS = 2
TAILQS = (1, 2)  # queue per chunk

# weight-pack layout: every head has 5 blocks (q, q_swap, k, k_swap,
# v_folded); rotate-half comes from a second pair-swapped weights matmul
# (GPSIMD cannot touch PSUM on real HW, so SBUF round-trips are avoided).
W_COLS = 5 * HPC


def _wslot(h, which):
    return 5 * h + {"q": 0, "qs": 1, "k": 2, "ks": 3, "v": 4}[which]


def _build_nc():
    nc = bacc.Bacc()
    x_d = nc.declare_dram_parameter("x", [128, NT, 128], F16, isOutput=False)
    wqkv_d = nc.declare_dram_parameter("wqkv", [128, W_COLS * 128], F16, isOutput=False)
    # packed [cos_b0 | sin_b0 | cos_b1 | sin_b1], each [128, WQ]
    cossin_d = nc.declare_dram_parameter("cossin", [128, 4 * WQ], F16, isOutput=False)
    identones_d = nc.declare_dram_parameter("identones", [128, 256], F16, isOutput=False)
    yt_d = nc.declare_dram_parameter("yt", [128, N], F16, isOutput=True)
    if DEBUG_DUMPS:
        dbg = {name: nc.declare_dram_parameter(f"dbg_{name}", [128, N], F16,
                                               isOutput=True)
               for name in ("xn", "xnT", "qh0", "kh0", "vh0", "e0", "r0")}

    with ExitStack() as ctx:
        tc = ctx.enter_context(tile.TileContext(nc))
        const = ctx.enter_context(tc.tile_pool(name="const", bufs=1))
        sq_p = ctx.enter_context(tc.tile_pool(name="sq", bufs=3))
        raw_p = ctx.enter_context(tc.tile_pool(name="raw", bufs=4))
        swp_p = ctx.enter_context(tc.tile_pool(name="swp", bufs=4))
        t12_p = ctx.enter_context(tc.tile_pool(name="t12", bufs=8))
        qh_p = ctx.enter_context(tc.tile_pool(name="qh", bufs=4))
        vh_p = ctx.enter_context(tc.tile_pool(name="vh", bufs=2))
        e_p = ctx.enter_context(tc.tile_pool(name="exps", bufs=10))
        s1_p = ctx.enter_context(tc.tile_pool(name="s1", bufs=8))
        s2_p = ctx.enter_context(tc.tile_pool(name="s2", bufs=6))
        s34_p = ctx.enter_context(tc.tile_pool(name="s34", bufs=4))
        rsb_p = ctx.enter_context(tc.tile_pool(name="rsb", bufs=2))
        rv_p = ctx.enter_context(tc.tile_pool(name="rv", bufs=3))
        osb_p = ctx.enter_context(tc.tile_pool(name="osb", bufs=3))
        on_p = ctx.enter_context(tc.tile_pool(name="onorm", bufs=6))
        y_p = ctx.enter_context(tc.tile_pool(name="y", bufs=4))
        ps_sc = ctx.enter_context(tc.tile_pool(name="ps_sc", bufs=3, space="PSUM"))
        ps_av = ctx.enter_context(tc.tile_pool(name="ps_av", bufs=1, space="PSUM"))

        # ---------------- input DMAs ----------------
        # SP queue, startup-critical order: the first exp needs (in chain
        # order) x[0:1024] -> LN -> xnT, head-0 q/k weights, cos/sin block 0.
        # The late halves (wqkv rest, cos/sin block 1) issue from the Pool
        # queue AFTER the xn ops so they can't steal early HWDGE slots.
        xt_all = const.tile([128, NT, 128], F16, tag="xt", name="xt_all")
        nc.sync.dma_start(out=xt_all[:, 0:4, :], in_=x_d[:, 0:4, :])
        nc.sync.dma_start(out=xt_all[:, 4:8, :], in_=x_d[:, 4:8, :])
        identones_t = const.tile([128, 256], F16, tag="identones",
                                 name="identones_t")
        nc.sync.dma_start(out=identones_t, in_=identones_d[:, :])
        wqkv_t = const.tile([128, W_COLS * 128], F16, tag="wqkv", name="wqkv_t")
        nc.sync.dma_start(out=wqkv_t[:, 0:640], in_=wqkv_d[:, 0:640])
        nc.sync.dma_start(out=xt_all[:, 8:16, :], in_=x_d[:, 8:16, :])
        cossin_t = const.tile([128, 4 * WQ], F16, tag="cossin", name="cossin_t")
        nc.sync.dma_start(out=cossin_t[:, 0:2 * WQ], in_=cossin_d[:, 0:2 * WQ])
        nc.sync.dma_start(out=wqkv_t[:, 640:W_COLS * 128],
                          in_=wqkv_d[:, 640:W_COLS * 128])
        nc.sync.dma_start(out=cossin_t[:, 2 * WQ:4 * WQ],
                          in_=cossin_d[:, 2 * WQ:4 * WQ])
        ident_t = identones_t[:, 0:128]
        ones_t = identones_t[:, 128:256]

        def cosv(b):
            return cossin_t[:, b * 2 * WQ: b * 2 * WQ + WQ]

        def sinv(b):
            return cossin_t[:, b * 2 * WQ + WQ: (b + 1) * 2 * WQ]

        def W(h, which):
            return wqkv_t[:, ts(_wslot(h, which), 128)]

        def xtile(t):
            return xt_all[:, t, :]

        # ---------------- LayerNorm (per 8-tile half) ----------------
        st_sum = const.tile([128, NT], F32, tag="st_sum", name="st_sum")
        st_sq = const.tile([128, NT], F32, tag="st_sq", name="st_sq")
        mean = const.tile([128, NT], F32, tag="mean", name="mean")
        msq = const.tile([128, NT], F32, tag="msq", name="msq")
        lnv = const.tile([128, NT], F32, tag="lnv", name="lnv")
        istd = const.tile([128, NT], F32, tag="istd", name="istd")
        epsb = const.tile([128, 1], F32, tag="epsb", name="epsb")
        nc.vector.memset(epsb, EPS)
        xn = const.tile([128, N], F16, tag="xn", name="xn")
        xnT = const.tile([128, N], F16, tag="xnT", name="xnT")

        def ln_stats4(lo):
            """Raw sums for 4 token tiles in 3 wide DVE ops (3-D access
            pattern: reduce innermost d only)."""
            xg = xt_all[:, lo:lo + 4, :]
            nc.vector.tensor_reduce(
                out=st_sum[:, lo:lo + 4], in_=xg, axis=AX.X, op=ALU.add)
            sq4 = sq_p.tile([128, 4, 128], F16, tag="sq4", name=f"sq4_{lo}")
            nc.vector.tensor_mul(sq4, xg, xg)
            nc.vector.tensor_reduce(
                out=st_sq[:, lo:lo + 4], in_=sq4, axis=AX.X, op=ALU.add)

        def ln_stats_act(lo, hi):
            """Stats via the (startup-idle) ACT engine's free-axis
            accumulator; Identity/Square live in the Exp table set."""
            for t in range(lo, hi):
                scr = sq_p.tile([128, 128], F16, tag="ascr", name=f"as{t}")
                nc.scalar.activation(out=scr, in_=xtile(t), func=AF.Identity,
                                     accum_out=st_sum[:, t:t + 1])
                scr2 = sq_p.tile([128, 128], F16, tag="ascr", name=f"aq{t}")
                nc.scalar.activation(out=scr2, in_=xtile(t), func=AF.Square,
                                     accum_out=st_sq[:, t:t + 1])

        def ln_small(lo, hi):
            """mean/var/istd chain (small DVE ops) for a tile range."""
            sl = slice(lo, hi)
            nc.vector.tensor_scalar_mul(mean[:, sl], st_sum[:, sl], 1.0 / DIM)
            nc.vector.tensor_mul(msq[:, sl], mean[:, sl], mean[:, sl])
            nc.vector.scalar_tensor_tensor(
                out=lnv[:, sl], in0=st_sq[:, sl], scalar=1.0 / DIM,
                in1=msq[:, sl], op0=ALU.mult, op1=ALU.subtract)
            nc.vector.tensor_scalar_add(lnv[:, sl], lnv[:, sl], EPS)
            w = lnv[:, sl]
            nc.vector.reciprocal_approx_fast(out=w, in_=w)
            nc.vector.tensor_scalar(istd[:, sl], w, 0.5, 0.5, ALU.mult, ALU.add)
            tmp = sq_p.tile([128, NT], F32, tag="nt", name=f"nt{lo}")
            tr = tmp[:, sl]
            for _ in range(2):
                nc.vector.reciprocal_approx_fast(out=tr, in_=istd[:, sl])
                nc.vector.scalar_tensor_tensor(
                    out=tr, in0=tr, scalar=0.5, in1=w,
                    op0=ALU.mult, op1=ALU.mult)
                nc.vector.scalar_tensor_tensor(
                    out=istd[:, sl], in0=istd[:, sl], scalar=0.5, in1=tr,
                    op0=ALU.mult, op1=ALU.add)

        def ln_xn_steps(lo, hi, xnT_ps, xn_engine=None):
            eng = xn_engine or nc.gpsimd
            for t in range(lo, hi):
                eng.tensor_scalar(
                    xn[:, ts(t, 128)], xtile(t), mean[:, t:t + 1],
                    istd[:, t:t + 1], ALU.subtract, ALU.mult)
            yield 100
            for t in range(lo, hi):
                nc.tensor.transpose(
                    out=xnT_ps[:, ts(t % 8, 128)], in_=xn[:, ts(t, 128)],
                    identity=ident_t)
                if t == lo + 2:
                    yield 300
            base = (lo // 8) * WQ
            nc.vector.tensor_copy(
                xnT[:, base + (lo % 8) * 128: base + (hi % 8 or 8) * 128],
                xnT_ps[:, (lo % 8) * 128:(hi % 8 or 8) * 128])
            yield 250

        def ln_xn(lo, hi, xnT_ps, xn_engine=None):
            for _ in ln_xn_steps(lo, hi, xnT_ps, xn_engine):
                pass

        # keep the PE continuously busy through the LN wait so it reaches
        # full pstate before the first real matmuls (ramp: mid until 3us of
        # continuous execution). Warm matmuls use a memset tile so they are
        # not gated on any input DMA.
        warm = ps_sc.tile([128, 512], F16, tag="sc", name="warm")
        for i in range(N_WARM):
            nc.tensor.transpose(out=warm[:, 0:128], in_=ident_t,
                                identity=ident_t)

        xnT_ps0 = ps_sc.tile([128, WQ], F16, tag="sc", name="xnT_ps0")
        ln_stats4(0)
        ln_small(0, 4)
        ln_xn(0, 4, xnT_ps0, nc.vector)
        ln_stats4(4)
        ln_small(4, 8)
        ln_xn(4, 8, xnT_ps0, nc.vector)
        ln_stats_act(8, 12)      # ACT: idle until the first exp
        ln_stats_act(12, 16)
        qh_t, kh_t, vh_t = {}, {}, {}
        vh_pre = {}

        # ---------------- head-0 QKV (startup path, rot via matmul) ------
        def h0_qk_block_steps(which, wsw, dst, b):
            p_ps = ps_sc.tile([128, WQ], F32, tag="sc", name=f"h0p{which}{b}")
            pr_ps = ps_sc.tile([128, WQ], F32, tag="sc", name=f"h0r{which}{b}")
            for src_w in (which, wsw):
                dst_ps = p_ps if src_w == which else pr_ps
                for hf in range(WQ // 512):
                    nc.tensor.matmul(
                        out=dst_ps[:, ts(hf, 512)], lhsT=W(0, src_w),
                        rhs=xnT[:, b * WQ + hf * 512: b * WQ + (hf + 1) * 512],
                        start=True, stop=True, skip_group_check=True)
                    yield 430
            t1 = t12_p.tile([128, WQ], F16, tag="t12", name=f"h0t1{which}{b}")
            nc.vector.tensor_mul(t1, p_ps, cosv(b))
            yield 600
            t2 = t12_p.tile([128, WQ], F16, tag="t12", name=f"h0t2{which}{b}")
            nc.vector.tensor_mul(t2, pr_ps, sinv(b))
            yield 600
            nc.vector.tensor_add(dst[:, ts(b, WQ)], t1, t2)
            yield 300

        def h0_qk_first(qh0, kh0):
            """q+k block-0 in 512-col chunks (one PSUM pair live at a time).
            k hf1 (j-tiles 4-7) is deferred to the feed: jb0-3 only need
            k[0:512], and every exp needs the full q block."""
            for hf, which, wsw, dst in ((0, "q", "qs", qh0), (0, "k", "ks", kh0),
                                        (1, "q", "qs", qh0), (1, "k", "ks", kh0)):
                sl = slice(hf * 512, (hf + 1) * 512)
                p_ps = ps_sc.tile([128, 512], F32, tag="sc",
                                  name=f"h0p{which}{hf}")
                pr_ps = ps_sc.tile([128, 512], F32, tag="sc",
                                   name=f"h0r{which}{hf}")
                nc.tensor.matmul(out=p_ps, lhsT=W(0, which), rhs=xnT[:, sl],
                                 start=True, stop=True, skip_group_check=True)
                nc.tensor.matmul(out=pr_ps, lhsT=W(0, wsw), rhs=xnT[:, sl],
                                 start=True, stop=True, skip_group_check=True)
                t1 = t12_p.tile([128, 512], F16, tag="t12h0",
                                name=f"f1{which}{hf}")
                nc.vector.tensor_mul(t1, p_ps, cossin_t[:, sl])
                t2 = t12_p.tile([128, 512], F16, tag="t12h0",
                                name=f"f2{which}{hf}")
                nc.vector.tensor_mul(
                    t2, pr_ps, cossin_t[:, WQ + hf * 512: WQ + (hf + 1) * 512])
                nc.gpsimd.tensor_add(dst[:, sl], t1, t2)

        def h0_k1_steps(kh0):
            sl = slice(512, 1024)
            p_ps = ps_sc.tile([128, 512], F32, tag="sc", name="h0pk1")
            pr_ps = ps_sc.tile([128, 512], F32, tag="sc", name="h0rk1")
            nc.tensor.matmul(out=p_ps, lhsT=W(0, "k"), rhs=xnT[:, sl],
                             start=True, stop=True, skip_group_check=True)
            nc.tensor.matmul(out=pr_ps, lhsT=W(0, "ks"), rhs=xnT[:, sl],
                             start=True, stop=True, skip_group_check=True)
            yield 300
            t1 = t12_p.tile([128, 512], F16, tag="t12h0", name="f1k1d")
            nc.vector.tensor_mul(t1, p_ps, cossin_t[:, sl])
            yield 500
            t2 = t12_p.tile([128, 512], F16, tag="t12h0", name="f2k1d")
            nc.vector.tensor_mul(
                t2, pr_ps, cossin_t[:, WQ + 512: WQ + 1024])
            yield 500
            nc.vector.tensor_add(kh0[:, sl], t1, t2)
            yield 300

        def v_block_steps(h, vh, b, split=False):
            v_ps = ps_sc.tile([128, WQ], F32, tag="sc", name=f"v{h}{b}")
            for c in range(WQ // 128):
                nc.tensor.matmul(
                    out=v_ps[:, ts(c, 128)],
                    lhsT=xnT[:, b * WQ + c * 128: b * WQ + (c + 1) * 128],
                    rhs=W(h, "v"), start=True, stop=True,
                    skip_group_check=True)
                if c == WQ // 256:
                    yield 215
            if split:
                # early AV j-tiles only need the first v columns
                nc.vector.tensor_copy(vh[:, b * WQ: b * WQ + 128],
                                      v_ps[:, 0:128])
                nc.vector.tensor_copy(vh[:, b * WQ + 128: b * WQ + 512],
                                      v_ps[:, 128:512])
                nc.vector.tensor_copy(vh[:, b * WQ + 512: (b + 1) * WQ],
                                      v_ps[:, 512:WQ])
            else:
                nc.vector.tensor_copy(vh[:, ts(b, WQ)], v_ps)
            yield 100

        def prologue_steps():
            """Head-0 setup. Block-0 q/k/v are inline (startup path); the
            rest (LN tail for tiles 8-15, block-1 q/k/v) is fed into head-0's
            first attention block in small steps."""
            qh0 = qh_p.tile([128, N], F16, tag="qh", name="qh0")
            kh0 = qh_p.tile([128, N], F16, tag="qh", name="kh0")
            vh0 = vh_p.tile([128, N], F16, tag="vh", name="vh0")
            qh_t[0], kh_t[0], vh_t[0] = qh0, kh0, vh0
            h0_qk_first(qh0, kh0)
            for _ in v_block_steps(0, vh0, 0, split=True):
                pass
            yield "prologue-ready"
            ln_small(8, 16)
            yield 200
            xnT_ps1 = ps_sc.tile([128, WQ], F16, tag="sc", name="xnT_ps1")
            for st in ln_xn_steps(8, 12, xnT_ps1):
                yield st
            for st in ln_xn_steps(12, 16, xnT_ps1):
                yield st
            for st in qk_rope_block(0, "k", "ks", kh0, 1, nc.vector):
                yield st
            yield "b1-k-done"
            for st in v_block_steps(0, vh0, 1):
                yield st
            vh1 = vh_p.tile([128, N], F16, tag="vh", name="vh1")
            vh_pre[1] = vh1
            yield 200
            v_block_act(1, vh1, 0)
            yield 430
            v_block_act(1, vh1, 1)
            yield "b1-kv-done"
            for st in qk_rope_block(0, "q", "qs", qh0, 1, nc.vector):
                yield st
            yield "pgen-done"

        # ---------------- per-head QKV (rotate-half via matmul) ----------
        def qk_rope_block(h, which, wsw, dst, b, add_engine):
            """p/pr projections for one [WQ] block + rope: DVE multiplies
            evacuate PSUM, the all-SBUF f16 add runs on `add_engine`."""
            p_ps = ps_sc.tile([128, WQ], F32, tag="sc", name=f"p{h}{which}{b}")
            pr_ps = ps_sc.tile([128, WQ], F32, tag="sc", name=f"r{h}{which}{b}")
            for dst_ps, wname in ((p_ps, which), (pr_ps, wsw)):
                for hf in range(WQ // 512):
                    nc.tensor.matmul(
                        out=dst_ps[:, ts(hf, 512)], lhsT=W(h, wname),
                        rhs=xnT[:, b * WQ + hf * 512: b * WQ + (hf + 1) * 512],
                        start=True, stop=True, skip_group_check=True)
                yield 430
            sl = ts(b, WQ)
            t1 = t12_p.tile([128, WQ], F16, tag="t12", name=f"t1{h}{which}{b}")
            nc.vector.tensor_mul(t1, p_ps, cosv(b))
            yield 600
            t2 = t12_p.tile([128, WQ], F16, tag="t12", name=f"t2{h}{which}{b}")
            nc.vector.tensor_mul(t2, pr_ps, sinv(b))
            yield 600
            add_engine.tensor_add(dst[:, sl], t1, t2)
            yield 300

        def v_block_act(h, vh, b):
            """v projection with ACT-side PSUM evacuation (for windows where
            ACT is otherwise idle, i.e. before the exp stream starts)."""
            v_ps = ps_sc.tile([128, WQ], F32, tag="sc", name=f"v{h}{b}")
            for c in range(WQ // 128):
                nc.tensor.matmul(
                    out=v_ps[:, ts(c, 128)],
                    lhsT=xnT[:, b * WQ + c * 128: b * WQ + (c + 1) * 128],
                    rhs=W(h, "v"), start=True, stop=True,
                    skip_group_check=True)
            nc.scalar.copy(out=vh[:, ts(b, WQ)], in_=v_ps)

        def qkv_steps(h):
            qh = qh_p.tile([128, N], F16, tag="qh", name=f"qh{h}")
            kh = qh_p.tile([128, N], F16, tag="qh", name=f"kh{h}")
            for dst, which, wsw in ((qh, "q", "qs"), (kh, "k", "ks")):
                for b in range(NQB):
                    for st in qk_rope_block(h, which, wsw, dst, b,
                                            nc.gpsimd if ROPE_ADD_POOL else nc.vector):
                        yield st
            qh_t[h], kh_t[h] = qh, kh
            if h == 1:
                vh_t[h] = vh_pre[h]     # computed in the prologue
                return
            vh = vh_p.tile([128, N], F16, tag="vh", name=f"vh{h}")
            for b in range(NQB):
                for st in v_block_steps(h, vh, b):
                    yield st
            vh_t[h] = vh

        # ---------------- attention ----------------
        onb_t = {}
        pair_t = {}
        pre_scores = {}

        def attention_block(h, qb, feed, pending, drain_feed, finish_prev):
            qh, kh, vh = qh_t[h], kh_t[h], vh_t[h]
            qsl = qh[:, ts(qb, WQ)]
            last = (h == HPC - 1 and qb == NQB - 1)

            def scores(jb):
                s_ps = ps_sc.tile([128, WQ], F32, tag="sc", name=f"s{h}{qb}{jb}")
                for hf in range(WQ // 512):
                    nc.tensor.matmul(
                        out=s_ps[:, ts(hf, 512)], lhsT=kh[:, ts(jb, 128)],
                        rhs=qsl[:, ts(hf, 512)], start=True, stop=True,
                        skip_group_check=True)
                return s_ps

            o_acc = ps_av.tile([128, WQ], F32, tag="av", name=f"oacc{h}{qb}")
            if h == 0 and qb == 1:
                drain_feed("pgen-done")   # qh0 block-1 emitted before use
            if h == 0 and qb == 0:
                s0a = ps_sc.tile([128, 512], F32, tag="sc", name="s00a")
                nc.tensor.matmul(out=s0a, lhsT=kh[:, 0:128], rhs=qsl[:, 0:512],
                                 start=True, stop=True, skip_group_check=True)
                s0b = ps_sc.tile([128, 512], F32, tag="sc", name="s00b")
                nc.tensor.matmul(out=s0b, lhsT=kh[:, 0:128],
                                 rhs=qsl[:, 512:1024],
                                 start=True, stop=True, skip_group_check=True)
                s_tiles = {0: (s0a, s0b), 1: scores(1)}
            elif (h, qb) in pre_scores:
                s_tiles = pre_scores.pop((h, qb))
            else:
                s_tiles = {0: scores(0), 1: scores(1)}
            # the previous block's R chain runs AFTER this block's first two
            # scores so the exp stream never waits on the denominator matmuls
            if finish_prev is not None and FR_JB < 0:
                finish_prev()
            if not (h == 0 and qb == 0):
                for _ in range(BOUNDARY_FEED):
                    feed()
            es, s1s, s2s, s3s = [], [], [], []
            osb = None
            for jb in range(JT):
                if h == 0 and qb == 0 and jb == KDRAIN_JB:
                    drain_feed("b1-k-done")
                if h == 0 and qb == 0 and jb == DRAIN_JB:
                    # all remaining head-0 prologue (kh0/vh0/qh0 block-1)
                    # must be emitted before the instructions that read it
                    # (emission order is program order; a later write would
                    # read as WAR)
                    drain_feed("pgen-done")
                if jb + 2 < JT and jb + 2 not in s_tiles:
                    s_tiles[jb + 2] = scores(jb + 2)
                e = e_p.tile([128, WQ], F16, tag="expT", name=f"e{h}{qb}{jb}")
                s_in = s_tiles.pop(jb)
                if isinstance(s_in, tuple):
                    nc.scalar.activation(out=e[:, 0:512], in_=s_in[0],
                                         func=AF.Exp, scale=SCALE)
                    nc.scalar.activation(out=e[:, 512:1024], in_=s_in[1],
                                         func=AF.Exp, scale=SCALE)
                else:
                    nc.scalar.activation(out=e, in_=s_in, func=AF.Exp,
                                         scale=SCALE)
                es.append(e)
                if DEBUG_DUMPS and DEBUG_DUMPS != 2 and h == 0 and qb == 0 and jb < 2:
                    nc.sync.dma_start(out=dbg["e0"][:, ts(jb, WQ)], in_=e)
                if DEBUG_DUMPS == 2 and h == 1 and qb == 0 and jb in (5, 12):
                    nc.sync.dma_start(out=dbg["e0"][:, ts(int(jb == 12), WQ)], in_=e)
                if jb == FR_JB and finish_prev is not None:
                    finish_prev()
                if jb == PENDING_JB and pending is not None:
                    pending()
                feed()
                if h == 0 and qb == 0:
                    for _ in range(FEED_H0 - 1):
                        feed()
                if jb == JT - 1 and qb == 0 and PRE_SCORES:
                    # the PE otherwise idles on exp(15) before the final AV
                    # group; the sibling q-block's kh/qh are already live
                    qsl1 = qh[:, ts(1, WQ)]
                    pre = {}
                    for j2 in range(2):
                        sp = ps_sc.tile([128, WQ], F32, tag="sc",
                                        name=f"ps{h}1{j2}")
                        for hf in range(WQ // 512):
                            nc.tensor.matmul(
                                out=sp[:, ts(hf, 512)], lhsT=kh[:, ts(j2, 128)],
                                rhs=qsl1[:, ts(hf, 512)], start=True, stop=True,
                                skip_group_check=True)
                        pre[j2] = sp
                    pre_scores[(h, 1)] = pre
                for hf in range(WQ // 512):
                    nc.tensor.matmul(
                        out=o_acc[:, ts(hf, 512)], lhsT=vh[:, ts(jb, 128)],
                        rhs=e[:, ts(hf, 512)], start=(jb == 0),
                        stop=(jb == JT - 1), skip_group_check=True)
                if jb % 2 == 1:
                    if jb == JT - 1 and not last:
                        pass      # whole final tree column deferred to finish_R
                    else:
                        s1 = s1_p.tile([128, WQ], F16, tag="s1",
                                       name=f"s1_{h}{qb}{jb}")
                        if last and jb == JT - 1:
                            # final links in 512 halves: the R chain for half 0
                            # starts one link earlier after the last exp
                            for hf in range(WQ // 512):
                                nc.vector.tensor_add(s1[:, ts(hf, 512)],
                                                     es[-2][:, ts(hf, 512)],
                                                     es[-1][:, ts(hf, 512)])
                        else:
                            nc.vector.tensor_add(s1, es[-2], es[-1])
                        s1s.append(s1)
                if jb % 4 == 3:
                    if jb == JT - 1 and not last:
                        pass      # deferred to finish_R
                    else:
                        s2 = s2_p.tile([128, WQ], F16, tag="s2",
                                       name=f"s2_{h}{qb}{jb}")
                        eng = (nc.gpsimd if len(s2s) < L2_POOL_COUNT and
                               not last else nc.vector)
                        if last and jb == JT - 1:
                            for hf in range(WQ // 512):
                                eng.tensor_add(s2[:, ts(hf, 512)],
                                               s1s[-2][:, ts(hf, 512)],
                                               s1s[-1][:, ts(hf, 512)])
                        else:
                            eng.tensor_add(s2, s1s[-2], s1s[-1])
                        s2s.append(s2)
                    if not last and jb == 7:
                        # L3 first half on Pool (plenty of time mid-block)
                        s3 = s34_p.tile([128, WQ], F16, tag="s34",
                                        name=f"s3_{h}{qb}{jb}")
                        nc.gpsimd.tensor_add(s3, s2s[-2], s2s[-1])
                        s3s.append(s3)
                if jb == JT - 1:
                    # AV accumulator evacuation frees the single PSUM slot.
                    # Emitted AFTER the tree adds so the R chain isn't stuck
                    # behind the 1.2us copy on DVE; the last block uses ACT
                    # (idle by then) to keep the DVE finish chain clean.
                    osb = osb_p.tile([128, WQ], F32, tag="osb",
                                     name=f"osb{h}{qb}")
                    if last:
                        nc.scalar.copy(out=osb, in_=o_acc)
                    else:
                        nc.vector.tensor_copy(osb, o_acc)
            rinv = rv_p.tile([128, WQ], F32, tag="rinv", name=f"rinv{h}{qb}")
            if last:
                # R inline, per-half chain + reciprocal so the finish pipeline
                # (norm/pair/y/dma) starts on half 0 while half 1 reduces
                R_ps = ps_sc.tile([128, WQ], F32, tag="sc", name=f"R{h}{qb}")
                for hf in range(WQ // 512):
                    for u, s2 in enumerate(s2s):
                        nc.tensor.matmul(
                            out=R_ps[:, ts(hf, 512)], lhsT=ones_t,
                            rhs=s2[:, ts(hf, 512)], start=(u == 0),
                            stop=(u == len(s2s) - 1), skip_group_check=True)
                    nc.vector.reciprocal_approx_fast(
                        out=rinv[:, ts(hf, 512)], in_=R_ps[:, ts(hf, 512)])

            def finish_R():
                # deferred into the next block (FR_JB): the final tree column
                # (s1_15, s2_3, s3b) runs here too, far from the boundary
                s1t = s1_p.tile([128, WQ], F16, tag="s1", name=f"s1t{h}{qb}")
                nc.vector.tensor_add(s1t, es[-2], es[-1])
                s2t = s2_p.tile([128, WQ], F16, tag="s2", name=f"s2t{h}{qb}")
                nc.vector.tensor_add(s2t, s1s[-1], s1t)
                s3t = s34_p.tile([128, WQ], F16, tag="s34", name=f"s3t{h}{qb}")
                nc.vector.tensor_add(s3t, s2s[-1], s2t)
                R_ps = ps_sc.tile([128, WQ], F32, tag="sc", name=f"R{h}{qb}")
                for u, s3 in enumerate((s3s[0], s3t)):
                    for hf in range(WQ // 512):
                        nc.tensor.matmul(
                            out=R_ps[:, ts(hf, 512)], lhsT=ones_t,
                            rhs=s3[:, ts(hf, 512)], start=(u == 0),
                            stop=(u == 1), skip_group_check=True)
                nc.vector.reciprocal_approx_fast(out=rinv, in_=R_ps)

            def tail():
                onb = on_p.tile([128, WQ], F16, tag="onb", name=f"onb{h}{qb}")
                nc.gpsimd.tensor_mul(onb, osb, rinv)
                onb_t[(h, qb)] = onb
                if h == 1:
                    pr = y_p.tile([128, WQ], F16, tag="pair", name=f"pr1{qb}")
                    nc.gpsimd.tensor_add(pr, onb_t[(0, qb)], onb_t[(1, qb)])
                    pair_t[(0, qb)] = pr
                if h == 2:
                    # pre-sum heads 0..2 so the final y is one add per chunk
                    pr = y_p.tile([128, WQ], F16, tag="pair", name=f"pre3{qb}")
                    nc.gpsimd.tensor_add(pr, pair_t[(0, qb)], onb_t[(2, qb)])
                    pair_t[(2, qb)] = pr
                if h == 3:
                    emit_y(qb, 1)

            if last:
                # low-latency all-DVE finish, fully chunked per 512
                onb = on_p.tile([128, WQ], F16, tag="onb", name=f"onb{h}{qb}")
                onb_t[(h, qb)] = onb
                y_sb = y_p.tile([128, WQ], F16, tag="ysb", name=f"ysb{qb}")
                for hf in range(WQ // 512):
                    sl = slice(hf * 512, (hf + 1) * 512)
                    nc.vector.tensor_mul(onb[:, sl], osb[:, sl], rinv[:, sl])
                    nc.vector.tensor_add(y_sb[:, sl], pair_t[(2, qb)][:, sl],
                                         onb[:, sl])
                    # spread the final DMAs across queues so their DGE
                    # generations run in parallel, not serialized on one path
                    for qtr in range(TAIL_CHUNKS // 2):
                        c = hf * (TAIL_CHUNKS // 2) + qtr
                        w = WQ // TAIL_CHUNKS
                        csl = slice(c * w, (c + 1) * w)
                        eng = (nc.scalar, nc.gpsimd, nc.sync,
                               nc.vector)[TAILQS[c]]
                        eng.dma_start(out=yt_d[:, qb * WQ + c * w:
                                               qb * WQ + (c + 1) * w],
                                      in_=y_sb[:, csl])
                return None, None
            return finish_R, tail

        def emit_y(qb, chunks):
            y_sb = y_p.tile([128, WQ], F16, tag="ysb", name=f"ysb{qb}")
            for hf in range(chunks):
                w = WQ // chunks
                sl = slice(hf * w, (hf + 1) * w)
                nc.vector.tensor_add(y_sb[:, sl], pair_t[(2, qb)][:, sl],
                                     onb_t[(3, qb)][:, sl])
                (nc.sync if EMITY_SP else nc.gpsimd).dma_start(
                    out=yt_d[:, qb * WQ + hf * w: qb * WQ + (hf + 1) * w],
                    in_=y_sb[:, sl])

        def run_head(h, feed, pending, drain_feed, finish_prev):
            for qb in range(NQB):
                finish_prev, pending = attention_block(
                    h, qb, feed, pending, drain_feed, finish_prev)
                feed_gate[0] = None   # qb0 of head 0 feeds the prologue only
            return finish_prev, pending

        # prologue: run until head-0 (qb0) inputs exist
        pgen = prologue_steps()
        next(pgen)
        if DEBUG_DUMPS == 2:
            pass  # keep interleaved schedule; dumps added at end
        elif DEBUG_DUMPS:
            for _ in pgen:
                pass
            nc.sync.dma_start(out=dbg["xn"][:, :], in_=xn)
            nc.sync.dma_start(out=dbg["xnT"][:, :], in_=xnT)
            nc.sync.dma_start(out=dbg["qh0"][:, :], in_=qh_t[0])
            nc.sync.dma_start(out=dbg["kh0"][:, :], in_=kh_t[0])
            nc.sync.dma_start(out=dbg["vh0"][:, :], in_=vh_t[0])

        pending = None
        finishR = None
        feed_gate = [None]   # when set, only this generator may be pulled
        for h in range(HPC):
            gens = []
            if h == 0:
                gens.append(pgen)
                feed_gate[0] = pgen
            if h + 1 < HPC:
                gens.append(qkv_steps(h + 1))
            if h == 0 and DRAIN_FIRST:
                for _ in range(DRAIN_FIRST):
                    try:
                        next(gens[0])
                    except StopIteration:
                        break
            if DRAIN == 2 or (DRAIN == 1 and h == 0):
                while gens:
                    try:
                        next(gens[0])
                    except StopIteration:
                        gens.pop(0)

            def feed(gs=gens):
                while gs:
                    try:
                        next(gs[0])
                        return
                    except StopIteration:
                        gs.pop(0)

            def drain_feed(marker=None, gs=gens):
                while gs:
                    try:
                        st = next(gs[0])
                        if marker is not None and st == marker:
                            return
                    except StopIteration:
                        gs.pop(0)

            finishR, pending = run_head(h, feed, pending, drain_feed, finishR)
            while gens:
                try:
                    next(gens[0])
                except StopIteration:
                    gens.pop(0)
        if pending is not None:
            pending()
        if DEBUG_DUMPS == 2:
            nc.sync.dma_start(out=dbg["xnT"][:, 0:WQ], in_=onb_t[(1, 0)])
            nc.sync.dma_start(out=dbg["xn"][:, :], in_=qh_t[1])
            nc.sync.dma_start(out=dbg["xnT"][:, :], in_=kh_t[1])
            nc.sync.dma_start(out=dbg["qh0"][:, :], in_=vh_t[1])
            nc.sync.dma_start(out=dbg["kh0"][:, :], in_=qh_t[2])
            nc.sync.dma_start(out=dbg["vh0"][:, :], in_=vh_t[2])
            nc.sync.dma_start(out=dbg["e0"][:, :], in_=qh_t[3])
            nc.sync.dma_start(out=dbg["r0"][:, :], in_=vh_t[3])

    nc.finalize()
    return nc


def _make_runner(nc, n_cores=8):
    """Cached jitted multi-core executor (mirrors bass2jax.run_bass_via_pjrt,
    minus output-donation so it can be called repeatedly for timing)."""
    import jax
    import jax.numpy as jnp
    from jax.sharding import Mesh, PartitionSpec
    from jax.experimental.shard_map import shard_map
    from concourse import bass2jax, mybir as mb
    bass2jax.install_neuronx_cc_hook()

    partition_name = nc.partition_id_tensor.name if nc.partition_id_tensor else None
    in_names, out_names, out_avals, zero_outs = [], [], [], []
    for alloc in nc.m.functions[0].allocations:
        if not isinstance(alloc, mb.MemoryLocationSet):
            continue
        name = alloc.memorylocations[0].name
        if alloc.kind == "ExternalInput":
            if name != partition_name:
                in_names.append(name)
        elif alloc.kind == "ExternalOutput":
            out_names.append(name)
            shape = tuple(alloc.tensor_shape)
            dtype = mb.dt.np(alloc.dtype)
            out_avals.append(jax.core.ShapedArray(shape, dtype))
            zero_outs.append(np.zeros(shape, dtype))
    n_params = len(in_names)
    all_in_names = list(in_names) + list(out_names)
    if partition_name is not None:
        all_in_names.append(partition_name)

    def _body(*args):
        operands = list(args)
        if partition_name is not None:
            operands.append(bass2jax.partition_id_tensor())
        outs = bass2jax._bass_exec_p.bind(
            *operands,
            out_avals=tuple(out_avals),
            in_names=tuple(all_in_names),
            out_names=tuple(out_names),
            lowering_input_output_aliases=(),
            sim_require_finite=True,
            sim_require_nnan=True,
            nc=nc,
        )
        return tuple(outs)

    devices = jax.devices()[:n_cores]
    mesh = Mesh(np.asarray(devices), ("core",))
    in_specs = (PartitionSpec("core"),) * (n_params + len(out_names))
    out_specs = (PartitionSpec("core"),) * len(out_names)
    donate = tuple(range(n_params, n_params + len(out_names)))
    sharded = jax.jit(shard_map(_body, mesh=mesh, in_specs=in_specs,
                                out_specs=out_specs, check_rep=False),
                      donate_argnums=donate, keep_unused=True)

    def run(in_maps):
        concat_in = [np.concatenate([np.asarray(in_maps[c][k]) for c in range(n_cores)], axis=0)
                     for k in in_names]
        concat_zero = [np.concatenate([z] * n_cores, axis=0) for z in zero_outs]
        outs = sharded(*concat_in, *concat_zero)
        outs = [np.asarray(o) for o in outs]
        res = []
        for c in range(n_cores):
            d = {}
            for i, name in enumerate(out_names):
                per = outs[i].shape[0] // n_cores
                d[name] = outs[i][c * per:(c + 1) * per]
            res.append(d)
        return res, sharded, (in_names, zero_outs)

    return run


def _rope_tables():
    """cos/sin tables in [d, n] layout; token N-1 unrotated; sin sign-folded."""
    inv_freq = 1.0 / (10000.0 ** (np.arange(0, HEAD, 2, dtype=np.float64) / HEAD))
    pos = np.arange(N, dtype=np.float64)
    ang = pos[None, :] * np.repeat(inv_freq, 2)[:, None]        # [d, n]
    cos_t = np.cos(ang)
    sin_t = np.sin(ang)
    sign = np.where(np.arange(HEAD) % 2 == 0, -1.0, 1.0)[:, None]
    sin_t = sin_t * sign
    cos_t[:, N - 1] = 1.0
    sin_t[:, N - 1] = 0.0
    return cos_t.astype(F16_NP), sin_t.astype(F16_NP)


def _prep_core_inputs(x, ln_gamma, ln_beta, w_qkv, w_out):
    """Build the 8 per-core input maps (host-side layout/packing)."""
    cos_t, sin_t = _rope_tables()
    ident = np.eye(128, dtype=np.float32)
    swap = np.arange(HEAD) ^ 1

    # packed [cos_b0 | sin_b0 | cos_b1 | sin_b1]
    cossin = np.concatenate(
        [cos_t[:, 0:WQ], sin_t[:, 0:WQ], cos_t[:, WQ:N], sin_t[:, WQ:N]],
        axis=1).astype(F16_NP)
    identones = np.concatenate(
        [ident.astype(F16_NP), np.ones((128, 128), dtype=F16_NP)], axis=1)

    in_maps = []
    for c in range(8):
        b = c % 4
        g = c // 4
        wq_blocks = []
        for i in range(HPC):
            h = g * HPC + i
            Wq = w_qkv[h * HEAD:(h + 1) * HEAD, :] * ln_gamma[None, :]
            Wk = w_qkv[INNER + h * HEAD:INNER + (h + 1) * HEAD, :] * ln_gamma[None, :]
            Wv = w_qkv[2 * INNER + h * HEAD:2 * INNER + (h + 1) * HEAD, :] * ln_gamma[None, :]
            Wo = w_out[:, h * HEAD:(h + 1) * HEAD]
            Wvo = Wo @ Wv                                        # fold out-proj into V
            wq_blocks += [Wq.T, Wq[swap, :].T, Wk.T, Wk[swap, :].T, Wvo.T]
        wqkv_packed = np.concatenate(wq_blocks, axis=1)          # [128, W_COLS*128]
        # pre-tile x to [128, NT, 128]: [p, t, d] holds x[b, t*128+p, d]
        xb = np.ascontiguousarray(
            x[b].reshape(NT, 128, DIM).transpose(1, 0, 2),
            dtype=F16_NP)
        in_maps.append({
            "x": xb,
            "wqkv": wqkv_packed.astype(F16_NP),
            "cossin": cossin,
            "identones": identones,
        })
    return in_maps


def kernel(x, ln_gamma, ln_beta, w_qkv, w_out, b_out):
    x = np.asarray(x, dtype=np.float32)
    ln_gamma = np.asarray(ln_gamma, dtype=np.float32)
    ln_beta = np.asarray(ln_beta, dtype=np.float32)
    w_qkv = np.asarray(w_qkv, dtype=np.float32)
    w_out = np.asarray(w_out, dtype=np.float32)
    b_out = np.asarray(b_out, dtype=np.float32)
    assert np.allclose(ln_beta, 0.0), "beta folding not implemented"

    if "nc" not in _CACHE:
        _CACHE["nc"] = _build_nc()
    nc = _CACHE["nc"]

    in_maps = _prep_core_inputs(x, ln_gamma, ln_beta, w_qkv, w_out)
    _CACHE["last_in_maps"] = in_maps
    res = run_bass_kernel_spmd(nc, in_maps, list(range(8)))
    results = res.results

    out = np.empty((B, N, DIM), dtype=np.float32)
    for b in range(B):
        y0 = np.asarray(results[b]["yt"], dtype=np.float32)
        y1 = np.asarray(results[b + 4]["yt"], dtype=np.float32)
        out[b] = (y0 + y1).T + b_out[None, :]
    return out



# revision 81
# speedup vs baseline: 1.0045x; 1.0015x over previous
"""Trainium2 Bass kernel for fused LN + QKV + partial-RoPE attention + out-proj.

Sharding: 8 cores = 4 batches x 2 head-groups (4 heads each).
Core c: batch = c % 4, heads = [4*(c//4) .. 4*(c//4)+3].
Each core returns a partial y^T [DIM, N] (f16); host sums the two
head-group partials per batch and adds b_out.

Design (173.8us; vs 186.5us prior / 265us original):
  - Dual bottleneck: ACT exp (~137us) and PE matmuls (~140us) are both
    near the wall; every change must keep both streaming.
  - exp in [128,1024] tiles (2 matmuls fill a 2-bank PSUM tile) to
    amortize ACT's fixed per-instruction access latency.
  - Softmax denominator: f16 pair-sum tree (L1 DVE, L2 Pool, L3 Pool)
    with the ENTIRE final tree column (s1_15/s2_3/s3b) plus the 2-link
    ones-matmul R chain deferred into the NEXT block at jb=FR_JB(3);
    by then the inputs are long ready so neither the exp stream nor the
    PE queue ever waits on denominator work. Deferring R to the block
    boundary alone (instead of jb3) costs ~3us.
  - AV accumulator evacuated by a DVE copy (after the tree adds) - not
    ACT - so the exp stream never pays for it; the last block evacuates
    on ACT (idle by then) to keep the DVE finish chain clean.
  - RoPE rotate-half via a second pair-swapped weights matmul for all
    heads (q, qs, k, ks, v_folded = 5 weight blocks/head; w_out is
    folded into V so AV directly yields y^T partials).
  - Startup (first exp ~15.3us): x loaded as f16 [128,NT,128] with the
    8 startup tiles in the first DMA; SP issues DMAs in dependency
    order (x0:8, identones, w-head0, cos/sin b0, x8:16, rest); LN raw
    sums for tiles 0-7 as fused 4-tile 3-D reduces on DVE, tiles 8-15
    on the startup-idle ACT via Identity/Square accum; istd via a short
    Newton chain on DVE; head-0 q/k/v in 512-col chunks ordered q-hf0,
    k-hf0, q-hf1, k-hf1 with jb0's scores+exp split into 512 halves so
    ACT starts as early as possible; ident-transpose warmup keeps the
    PE ramped through the LN wait.
  - Per-head QKV+rope work for head h+1 is interleaved into the emission
    stream of head h's attention in <=512-col steps.
  - y DMAs issue from the Pool queue (cheap dispatch, SP stays free).
  - PSUM: scores pool 3x[128,1024]f32 (6 banks, also serves QKV/xnT/R
    tiles) + single AV accumulator (2 banks). This is exactly full; a
    manual single-tile arena or wider exp groups do NOT fit/pay (tried:
    subtile-dep serialization, pair-WAR stalls).
"""

import numpy as np
import ml_dtypes
from contextlib import ExitStack

import concourse.bass as bass
import concourse.tile as tile
from concourse import bacc
from concourse import mybir
from concourse import bass_isa
from concourse.bass import ts
from concourse.bass_utils import run_bass_kernel_spmd

B, N, DIM = 4, 2048, 128
HEADS, HEAD = 8, 128
INNER = HEADS * HEAD
HPC = 4            # heads per core
NT = N // 128      # 16 token tiles
WQ = 1024          # attention q-block width
NQB = N // WQ      # 2 q-blocks per head
JT = 16            # j sub-blocks (128 each) per q-block
EPS = 1e-5
SCALE = HEAD ** -0.5

F32 = mybir.dt.float32
F16 = mybir.dt.float16
AF = mybir.ActivationFunctionType
ALU = mybir.AluOpType
AX = mybir.AxisListType

F16_NP = np.float16

_CACHE = {}
DEBUG_DUMPS = False
DRAIN = 0
DRAIN_FIRST = 0
ROPE_ADD_POOL = True
L2_POOL_COUNT = 3
N_WARM = 24
PENDING_JB = 5
FEED_H0 = 2
BOUNDARY_FEED = 0
S1_POOL_PAR = 2  # 2 = never on pool
PRE_SCORES = True
FR_JB = 3
FR_DEPTH4 = False
R_PRERUN = True
R_PRE_JB = 13
DRAIN_JB = 6
KDRAIN_JB = 4
EMITY_SP = False
TAIL_CHUNKS = 2
TAIL_ORDER = None
TAILQS = (1, 2)  # queue per chunk

# weight-pack layout: every head has 5 blocks (q, q_swap, k, k_swap,
# v_folded); rotate-half comes from a second pair-swapped weights matmul
# (GPSIMD cannot touch PSUM on real HW, so SBUF round-trips are avoided).
W_COLS = 5 * HPC


def _wslot(h, which):
    return 5 * h + {"q": 0, "qs": 1, "k": 2, "ks": 3, "v": 4}[which]


def _build_nc():
    nc = bacc.Bacc()
    x_d = nc.declare_dram_parameter("x", [128, NT, 128], F16, isOutput=False)
    wqkv_d = nc.declare_dram_parameter("wqkv", [128, W_COLS * 128], F16, isOutput=False)
    # packed [cos_b0 | sin_b0 | cos_b1 | sin_b1], each [128, WQ]
    cossin_d = nc.declare_dram_parameter("cossin", [128, 4 * WQ], F16, isOutput=False)
    identones_d = nc.declare_dram_parameter("identones", [128, 256], F16, isOutput=False)
    yt_d = nc.declare_dram_parameter("yt", [128, N], F16, isOutput=True)
    if DEBUG_DUMPS:
        dbg = {name: nc.declare_dram_parameter(f"dbg_{name}", [128, N], F16,
                                               isOutput=True)
               for name in ("xn", "xnT", "qh0", "kh0", "vh0", "e0", "r0")}

    with ExitStack() as ctx:
        tc = ctx.enter_context(tile.TileContext(nc))
        const = ctx.enter_context(tc.tile_pool(name="const", bufs=1))
        sq_p = ctx.enter_context(tc.tile_pool(name="sq", bufs=3))
        raw_p = ctx.enter_context(tc.tile_pool(name="raw", bufs=4))
        swp_p = ctx.enter_context(tc.tile_pool(name="swp", bufs=4))
        t12_p = ctx.enter_context(tc.tile_pool(name="t12", bufs=8))
        qh_p = ctx.enter_context(tc.tile_pool(name="qh", bufs=4))
        vh_p = ctx.enter_context(tc.tile_pool(name="vh", bufs=2))
        e_p = ctx.enter_context(tc.tile_pool(name="exps", bufs=10))
        s1_p = ctx.enter_context(tc.tile_pool(name="s1", bufs=8))
        s2_p = ctx.enter_context(tc.tile_pool(name="s2", bufs=6))
        s34_p = ctx.enter_context(tc.tile_pool(name="s34", bufs=4))
        rsb_p = ctx.enter_context(tc.tile_pool(name="rsb", bufs=2))
        rv_p = ctx.enter_context(tc.tile_pool(name="rv", bufs=3))
        osb_p = ctx.enter_context(tc.tile_pool(name="osb", bufs=3))
        on_p = ctx.enter_context(tc.tile_pool(name="onorm", bufs=6))
        y_p = ctx.enter_context(tc.tile_pool(name="y", bufs=4))
        ps_sc = ctx.enter_context(tc.tile_pool(name="ps_sc", bufs=3, space="PSUM"))
        ps_av = ctx.enter_context(tc.tile_pool(name="ps_av", bufs=1, space="PSUM"))

        # ---------------- input DMAs ----------------
        # SP queue, startup-critical order: the first exp needs (in chain
        # order) x[0:1024] -> LN -> xnT, head-0 q/k weights, cos/sin block 0.
        # The late halves (wqkv rest, cos/sin block 1) issue from the Pool
        # queue AFTER the xn ops so they can't steal early HWDGE slots.
        xt_all = const.tile([128, NT, 128], F16, tag="xt", name="xt_all")
        nc.sync.dma_start(out=xt_all[:, 0:4, :], in_=x_d[:, 0:4, :])
        nc.sync.dma_start(out=xt_all[:, 4:8, :], in_=x_d[:, 4:8, :])
        identones_t = const.tile([128, 256], F16, tag="identones",
                                 name="identones_t")
        nc.sync.dma_start(out=identones_t, in_=identones_d[:, :])
        wqkv_t = const.tile([128, W_COLS * 128], F16, tag="wqkv", name="wqkv_t")
        nc.sync.dma_start(out=wqkv_t[:, 0:640], in_=wqkv_d[:, 0:640])
        nc.sync.dma_start(out=xt_all[:, 8:16, :], in_=x_d[:, 8:16, :])
        cossin_t = const.tile([128, 4 * WQ], F16, tag="cossin", name="cossin_t")
        nc.sync.dma_start(out=cossin_t[:, 0:2 * WQ], in_=cossin_d[:, 0:2 * WQ])
        nc.sync.dma_start(out=wqkv_t[:, 640:W_COLS * 128],
                          in_=wqkv_d[:, 640:W_COLS * 128])
        nc.sync.dma_start(out=cossin_t[:, 2 * WQ:4 * WQ],
                          in_=cossin_d[:, 2 * WQ:4 * WQ])
        ident_t = identones_t[:, 0:128]
        ones_t = identones_t[:, 128:256]

        def cosv(b):
            return cossin_t[:, b * 2 * WQ: b * 2 * WQ + WQ]

        def sinv(b):
            return cossin_t[:, b * 2 * WQ + WQ: (b + 1) * 2 * WQ]

        def W(h, which):
            return wqkv_t[:, ts(_wslot(h, which), 128)]

        def xtile(t):
            return xt_all[:, t, :]

        # ---------------- LayerNorm (per 8-tile half) ----------------
        st_sum = const.tile([128, NT], F32, tag="st_sum", name="st_sum")
        st_sq = const.tile([128, NT], F32, tag="st_sq", name="st_sq")
        mean = const.tile([128, NT], F32, tag="mean", name="mean")
        msq = const.tile([128, NT], F32, tag="msq", name="msq")
        lnv = const.tile([128, NT], F32, tag="lnv", name="lnv")
        istd = const.tile([128, NT], F32, tag="istd", name="istd")
        epsb = const.tile([128, 1], F32, tag="epsb", name="epsb")
        nc.vector.memset(epsb, EPS)
        xn = const.tile([128, N], F16, tag="xn", name="xn")
        xnT = const.tile([128, N], F16, tag="xnT", name="xnT")

        def ln_stats4(lo):
            """Raw sums for 4 token tiles in 3 wide DVE ops (3-D access
            pattern: reduce innermost d only)."""
            xg = xt_all[:, lo:lo + 4, :]
            nc.vector.tensor_reduce(
                out=st_sum[:, lo:lo + 4], in_=xg, axis=AX.X, op=ALU.add)
            sq4 = sq_p.tile([128, 4, 128], F16, tag="sq4", name=f"sq4_{lo}")
            nc.vector.tensor_mul(sq4, xg, xg)
            nc.vector.tensor_reduce(
                out=st_sq[:, lo:lo + 4], in_=sq4, axis=AX.X, op=ALU.add)

        def ln_stats_act(lo, hi):
            """Stats via the (startup-idle) ACT engine's free-axis
            accumulator; Identity/Square live in the Exp table set."""
            for t in range(lo, hi):
                scr = sq_p.tile([128, 128], F16, tag="ascr", name=f"as{t}")
                nc.scalar.activation(out=scr, in_=xtile(t), func=AF.Identity,
                                     accum_out=st_sum[:, t:t + 1])
                scr2 = sq_p.tile([128, 128], F16, tag="ascr", name=f"aq{t}")
                nc.scalar.activation(out=scr2, in_=xtile(t), func=AF.Square,
                                     accum_out=st_sq[:, t:t + 1])

        def ln_small(lo, hi):
            """mean/var/istd chain (small DVE ops) for a tile range."""
            sl = slice(lo, hi)
            nc.vector.tensor_scalar_mul(mean[:, sl], st_sum[:, sl], 1.0 / DIM)
            nc.vector.tensor_mul(msq[:, sl], mean[:, sl], mean[:, sl])
            nc.vector.scalar_tensor_tensor(
                out=lnv[:, sl], in0=st_sq[:, sl], scalar=1.0 / DIM,
                in1=msq[:, sl], op0=ALU.mult, op1=ALU.subtract)
            nc.vector.tensor_scalar_add(lnv[:, sl], lnv[:, sl], EPS)
            w = lnv[:, sl]
            nc.vector.reciprocal_approx_fast(out=w, in_=w)
            nc.vector.tensor_scalar(istd[:, sl], w, 0.5, 0.5, ALU.mult, ALU.add)
            tmp = sq_p.tile([128, NT], F32, tag="nt", name=f"nt{lo}")
            tr = tmp[:, sl]
            for _ in range(2):
                nc.vector.reciprocal_approx_fast(out=tr, in_=istd[:, sl])
                nc.vector.scalar_tensor_tensor(
                    out=tr, in0=tr, scalar=0.5, in1=w,
                    op0=ALU.mult, op1=ALU.mult)
                nc.vector.scalar_tensor_tensor(
                    out=istd[:, sl], in0=istd[:, sl], scalar=0.5, in1=tr,
                    op0=ALU.mult, op1=ALU.add)

        def ln_xn_steps(lo, hi, xnT_ps, xn_engine=None):
            eng = xn_engine or nc.gpsimd
            for t in range(lo, hi):
                eng.tensor_scalar(
                    xn[:, ts(t, 128)], xtile(t), mean[:, t:t + 1],
                    istd[:, t:t + 1], ALU.subtract, ALU.mult)
            yield 100
            for t in range(lo, hi):
                nc.tensor.transpose(
                    out=xnT_ps[:, ts(t % 8, 128)], in_=xn[:, ts(t, 128)],
                    identity=ident_t)
                if t == lo + 2:
                    yield 300
            base = (lo // 8) * WQ
            nc.vector.tensor_copy(
                xnT[:, base + (lo % 8) * 128: base + (hi % 8 or 8) * 128],
                xnT_ps[:, (lo % 8) * 128:(hi % 8 or 8) * 128])
            yield 250

        def ln_xn(lo, hi, xnT_ps, xn_engine=None):
            for _ in ln_xn_steps(lo, hi, xnT_ps, xn_engine):
                pass

        # keep the PE continuously busy through the LN wait so it reaches
        # full pstate before the first real matmuls (ramp: mid until 3us of
        # continuous execution). Warm matmuls use a memset tile so they are
        # not gated on any input DMA.
        warm = ps_sc.tile([128, 512], F16, tag="sc", name="warm")
        for i in range(N_WARM):
            nc.tensor.transpose(out=warm[:, 0:128], in_=ident_t,
                                identity=ident_t)

        xnT_ps0 = ps_sc.tile([128, WQ], F16, tag="sc", name="xnT_ps0")
        ln_stats4(0)
        ln_small(0, 4)
        ln_xn(0, 4, xnT_ps0, nc.vector)
        ln_stats4(4)
        ln_small(4, 8)
        ln_xn(4, 8, xnT_ps0, nc.vector)
        ln_stats_act(8, 12)      # ACT: idle until the first exp
        ln_stats_act(12, 16)
        qh_t, kh_t, vh_t = {}, {}, {}
        vh_pre = {}

        # ---------------- head-0 QKV (startup path, rot via matmul) ------
        def h0_qk_block_steps(which, wsw, dst, b):
            p_ps = ps_sc.tile([128, WQ], F32, tag="sc", name=f"h0p{which}{b}")
            pr_ps = ps_sc.tile([128, WQ], F32, tag="sc", name=f"h0r{which}{b}")
            for src_w in (which, wsw):
                dst_ps = p_ps if src_w == which else pr_ps
                for hf in range(WQ // 512):
                    nc.tensor.matmul(
                        out=dst_ps[:, ts(hf, 512)], lhsT=W(0, src_w),
                        rhs=xnT[:, b * WQ + hf * 512: b * WQ + (hf + 1) * 512],
                        start=True, stop=True, skip_group_check=True)
                    yield 430
            t1 = t12_p.tile([128, WQ], F16, tag="t12", name=f"h0t1{which}{b}")
            nc.vector.tensor_mul(t1, p_ps, cosv(b))
            yield 600
            t2 = t12_p.tile([128, WQ], F16, tag="t12", name=f"h0t2{which}{b}")
            nc.vector.tensor_mul(t2, pr_ps, sinv(b))
            yield 600
            nc.vector.tensor_add(dst[:, ts(b, WQ)], t1, t2)
            yield 300

        def h0_qk_first(qh0, kh0):
            """q+k block-0 in 512-col chunks (one PSUM pair live at a time).
            k hf1 (j-tiles 4-7) is deferred to the feed: jb0-3 only need
            k[0:512], and every exp needs the full q block."""
            for hf, which, wsw, dst in ((0, "q", "qs", qh0), (0, "k", "ks", kh0),
                                        (1, "q", "qs", qh0), (1, "k", "ks", kh0)):
                sl = slice(hf * 512, (hf + 1) * 512)
                p_ps = ps_sc.tile([128, 512], F32, tag="sc",
                                  name=f"h0p{which}{hf}")
                pr_ps = ps_sc.tile([128, 512], F32, tag="sc",
                                   name=f"h0r{which}{hf}")
                nc.tensor.matmul(out=p_ps, lhsT=W(0, which), rhs=xnT[:, sl],
                                 start=True, stop=True, skip_group_check=True)
                nc.tensor.matmul(out=pr_ps, lhsT=W(0, wsw), rhs=xnT[:, sl],
                                 start=True, stop=True, skip_group_check=True)
                t1 = t12_p.tile([128, 512], F16, tag="t12h0",
                                name=f"f1{which}{hf}")
                nc.vector.tensor_mul(t1, p_ps, cossin_t[:, sl])
                t2 = t12_p.tile([128, 512], F16, tag="t12h0",
                                name=f"f2{which}{hf}")
                nc.vector.tensor_mul(
                    t2, pr_ps, cossin_t[:, WQ + hf * 512: WQ + (hf + 1) * 512])
                nc.gpsimd.tensor_add(dst[:, sl], t1, t2)

        def h0_k1_steps(kh0):
            sl = slice(512, 1024)
            p_ps = ps_sc.tile([128, 512], F32, tag="sc", name="h0pk1")
            pr_ps = ps_sc.tile([128, 512], F32, tag="sc", name="h0rk1")
            nc.tensor.matmul(out=p_ps, lhsT=W(0, "k"), rhs=xnT[:, sl],
                             start=True, stop=True, skip_group_check=True)
            nc.tensor.matmul(out=pr_ps, lhsT=W(0, "ks"), rhs=xnT[:, sl],
                             start=True, stop=True, skip_group_check=True)
            yield 300
            t1 = t12_p.tile([128, 512], F16, tag="t12h0", name="f1k1d")
            nc.vector.tensor_mul(t1, p_ps, cossin_t[:, sl])
            yield 500
            t2 = t12_p.tile([128, 512], F16, tag="t12h0", name="f2k1d")
            nc.vector.tensor_mul(
                t2, pr_ps, cossin_t[:, WQ + 512: WQ + 1024])
            yield 500
            nc.vector.tensor_add(kh0[:, sl], t1, t2)
            yield 300

        def v_block_steps(h, vh, b, split=False):
            v_ps = ps_sc.tile([128, WQ], F32, tag="sc", name=f"v{h}{b}")
            for c in range(WQ // 128):
                nc.tensor.matmul(
                    out=v_ps[:, ts(c, 128)],
                    lhsT=xnT[:, b * WQ + c * 128: b * WQ + (c + 1) * 128],
                    rhs=W(h, "v"), start=True, stop=True,
                    skip_group_check=True)
                if c == WQ // 256:
                    yield 215
            if split:
                # early AV j-tiles only need the first v columns
                nc.vector.tensor_copy(vh[:, b * WQ: b * WQ + 128],
                                      v_ps[:, 0:128])
                nc.vector.tensor_copy(vh[:, b * WQ + 128: b * WQ + 512],
                                      v_ps[:, 128:512])
                nc.vector.tensor_copy(vh[:, b * WQ + 512: (b + 1) * WQ],
                                      v_ps[:, 512:WQ])
            else:
                nc.vector.tensor_copy(vh[:, ts(b, WQ)], v_ps)
            yield 100

        def prologue_steps():
            """Head-0 setup. Block-0 q/k/v are inline (startup path); the
            rest (LN tail for tiles 8-15, block-1 q/k/v) is fed into head-0's
            first attention block in small steps."""
            qh0 = qh_p.tile([128, N], F16, tag="qh", name="qh0")
            kh0 = qh_p.tile([128, N], F16, tag="qh", name="kh0")
            vh0 = vh_p.tile([128, N], F16, tag="vh", name="vh0")
            qh_t[0], kh_t[0], vh_t[0] = qh0, kh0, vh0
            h0_qk_first(qh0, kh0)
            for _ in v_block_steps(0, vh0, 0, split=True):
                pass
            yield "prologue-ready"
            ln_small(8, 16)
            yield 200
            xnT_ps1 = ps_sc.tile([128, WQ], F16, tag="sc", name="xnT_ps1")
            for st in ln_xn_steps(8, 12, xnT_ps1):
                yield st
            for st in ln_xn_steps(12, 16, xnT_ps1):
                yield st
            for st in qk_rope_block(0, "k", "ks", kh0, 1, nc.vector):
                yield st
            yield "b1-k-done"
            for st in v_block_steps(0, vh0, 1):
                yield st
            vh1 = vh_p.tile([128, N], F16, tag="vh", name="vh1")
            vh_pre[1] = vh1
            yield 200
            v_block_act(1, vh1, 0)
            yield 430
            v_block_act(1, vh1, 1)
            yield "b1-kv-done"
            for st in qk_rope_block(0, "q", "qs", qh0, 1, nc.vector):
                yield st
            yield "pgen-done"

        # ---------------- per-head QKV (rotate-half via matmul) ----------
        def qk_rope_block(h, which, wsw, dst, b, add_engine):
            """p/pr projections for one [WQ] block + rope: DVE multiplies
            evacuate PSUM, the all-SBUF f16 add runs on `add_engine`."""
            p_ps = ps_sc.tile([128, WQ], F32, tag="sc", name=f"p{h}{which}{b}")
            pr_ps = ps_sc.tile([128, WQ], F32, tag="sc", name=f"r{h}{which}{b}")
            for dst_ps, wname in ((p_ps, which), (pr_ps, wsw)):
                for hf in range(WQ // 512):
                    nc.tensor.matmul(
                        out=dst_ps[:, ts(hf, 512)], lhsT=W(h, wname),
                        rhs=xnT[:, b * WQ + hf * 512: b * WQ + (hf + 1) * 512],
                        start=True, stop=True, skip_group_check=True)
                yield 430
            sl = ts(b, WQ)
            t1 = t12_p.tile([128, WQ], F16, tag="t12", name=f"t1{h}{which}{b}")
            nc.vector.tensor_mul(t1, p_ps, cosv(b))
            yield 600
            t2 = t12_p.tile([128, WQ], F16, tag="t12", name=f"t2{h}{which}{b}")
            nc.vector.tensor_mul(t2, pr_ps, sinv(b))
            yield 600
            add_engine.tensor_add(dst[:, sl], t1, t2)
            yield 300

        def v_block_act(h, vh, b):
            """v projection with ACT-side PSUM evacuation (for windows where
            ACT is otherwise idle, i.e. before the exp stream starts)."""
            v_ps = ps_sc.tile([128, WQ], F32, tag="sc", name=f"v{h}{b}")
            for c in range(WQ // 128):
                nc.tensor.matmul(
                    out=v_ps[:, ts(c, 128)],
                    lhsT=xnT[:, b * WQ + c * 128: b * WQ + (c + 1) * 128],
                    rhs=W(h, "v"), start=True, stop=True,
                    skip_group_check=True)
            nc.scalar.copy(out=vh[:, ts(b, WQ)], in_=v_ps)

        def qkv_steps(h):
            qh = qh_p.tile([128, N], F16, tag="qh", name=f"qh{h}")
            kh = qh_p.tile([128, N], F16, tag="qh", name=f"kh{h}")
            for dst, which, wsw in ((qh, "q", "qs"), (kh, "k", "ks")):
                for b in range(NQB):
                    for st in qk_rope_block(h, which, wsw, dst, b,
                                            nc.gpsimd if ROPE_ADD_POOL else nc.vector):
                        yield st
            qh_t[h], kh_t[h] = qh, kh
            if h == 1:
                vh_t[h] = vh_pre[h]     # computed in the prologue
                return
            vh = vh_p.tile([128, N], F16, tag="vh", name=f"vh{h}")
            for b in range(NQB):
                for st in v_block_steps(h, vh, b):
                    yield st
            vh_t[h] = vh

        # ---------------- attention ----------------
        onb_t = {}
        pair_t = {}
        pre_scores = {}

        def attention_block(h, qb, feed, pending, drain_feed, finish_prev):
            qh, kh, vh = qh_t[h], kh_t[h], vh_t[h]
            qsl = qh[:, ts(qb, WQ)]
            last = (h == HPC - 1 and qb == NQB - 1)

            def scores(jb):
                s_ps = ps_sc.tile([128, WQ], F32, tag="sc", name=f"s{h}{qb}{jb}")
                for hf in range(WQ // 512):
                    nc.tensor.matmul(
                        out=s_ps[:, ts(hf, 512)], lhsT=kh[:, ts(jb, 128)],
                        rhs=qsl[:, ts(hf, 512)], start=True, stop=True,
                        skip_group_check=True)
                return s_ps

            o_acc = ps_av.tile([128, WQ], F32, tag="av", name=f"oacc{h}{qb}")
            if h == 0 and qb == 1:
                drain_feed("pgen-done")   # qh0 block-1 emitted before use
            if h == 0 and qb == 0:
                s0a = ps_sc.tile([128, 512], F32, tag="sc", name="s00a")
                nc.tensor.matmul(out=s0a, lhsT=kh[:, 0:128], rhs=qsl[:, 0:512],
                                 start=True, stop=True, skip_group_check=True)
                s0b = ps_sc.tile([128, 512], F32, tag="sc", name="s00b")
                nc.tensor.matmul(out=s0b, lhsT=kh[:, 0:128],
                                 rhs=qsl[:, 512:1024],
                                 start=True, stop=True, skip_group_check=True)
                s_tiles = {0: (s0a, s0b), 1: scores(1)}
            elif (h, qb) in pre_scores:
                s_tiles = pre_scores.pop((h, qb))
            else:
                s_tiles = {0: scores(0), 1: scores(1)}
            # the previous block's R chain runs AFTER this block's first two
            # scores so the exp stream never waits on the denominator matmuls
            if finish_prev is not None and FR_JB < 0:
                finish_prev()
            if not (h == 0 and qb == 0):
                for _ in range(BOUNDARY_FEED):
                    feed()
            es, s1s, s2s, s3s = [], [], [], []
            R_pre = [] if R_PRERUN else None
            osb = None
            for jb in range(JT):
                if h == 0 and qb == 0 and jb == KDRAIN_JB:
                    drain_feed("b1-k-done")
                if h == 0 and qb == 0 and jb == DRAIN_JB:
                    # all remaining head-0 prologue (kh0/vh0/qh0 block-1)
                    # must be emitted before the instructions that read it
                    # (emission order is program order; a later write would
                    # read as WAR)
                    drain_feed("pgen-done")
                if jb + 2 < JT and jb + 2 not in s_tiles:
                    s_tiles[jb + 2] = scores(jb + 2)
                e = e_p.tile([128, WQ], F16, tag="expT", name=f"e{h}{qb}{jb}")
                s_in = s_tiles.pop(jb)
                if isinstance(s_in, tuple):
                    nc.scalar.activation(out=e[:, 0:512], in_=s_in[0],
                                         func=AF.Exp, scale=SCALE)
                    nc.scalar.activation(out=e[:, 512:1024], in_=s_in[1],
                                         func=AF.Exp, scale=SCALE)
                else:
                    nc.scalar.activation(out=e, in_=s_in, func=AF.Exp,
                                         scale=SCALE)
                es.append(e)
                if DEBUG_DUMPS and DEBUG_DUMPS != 2 and h == 0 and qb == 0 and jb < 2:
                    nc.sync.dma_start(out=dbg["e0"][:, ts(jb, WQ)], in_=e)
                if DEBUG_DUMPS == 2 and h == 1 and qb == 0 and jb in (5, 12):
                    nc.sync.dma_start(out=dbg["e0"][:, ts(int(jb == 12), WQ)], in_=e)
                if jb == FR_JB and finish_prev is not None:
                    finish_prev()
                if last and jb == R_PRE_JB and R_pre is not None:
                    # R links over the long-ready s2 tiles pre-run here; only
                    # the final link + recip stay on the post-exp15 tail
                    R_pre.append(ps_sc.tile([128, WQ], F32, tag="sc",
                                            name=f"Rp{h}{qb}"))
                    for hf in range(WQ // 512):
                        for u in range(3):
                            nc.tensor.matmul(
                                out=R_pre[-1][:, ts(hf, 512)], lhsT=ones_t,
                                rhs=s2s[u][:, ts(hf, 512)], start=(u == 0),
                                stop=False, skip_group_check=True)
                if jb == PENDING_JB and pending is not None:
                    pending()
                feed()
                if h == 0 and qb == 0:
                    for _ in range(FEED_H0 - 1):
                        feed()
                if jb == JT - 1 and qb == 0 and PRE_SCORES:
                    # the PE otherwise idles on exp(15) before the final AV
                    # group; the sibling q-block's kh/qh are already live
                    qsl1 = qh[:, ts(1, WQ)]
                    pre = {}
                    for j2 in range(2):
                        sp = ps_sc.tile([128, WQ], F32, tag="sc",
                                        name=f"ps{h}1{j2}")
                        for hf in range(WQ // 512):
                            nc.tensor.matmul(
                                out=sp[:, ts(hf, 512)], lhsT=kh[:, ts(j2, 128)],
                                rhs=qsl1[:, ts(hf, 512)], start=True, stop=True,
                                skip_group_check=True)
                        pre[j2] = sp
                    pre_scores[(h, 1)] = pre
                for hf in range(WQ // 512):
                    nc.tensor.matmul(
                        out=o_acc[:, ts(hf, 512)], lhsT=vh[:, ts(jb, 128)],
                        rhs=e[:, ts(hf, 512)], start=(jb == 0),
                        stop=(jb == JT - 1), skip_group_check=True)
                if jb % 2 == 1:
                    if jb == JT - 1 and not last:
                        pass      # whole final tree column deferred to finish_R
                    else:
                        s1 = s1_p.tile([128, WQ], F16, tag="s1",
                                       name=f"s1_{h}{qb}{jb}")
                        if last and jb == JT - 1:
                            # final links in 512 halves: the R chain for half 0
                            # starts one link earlier after the last exp
                            for hf in range(WQ // 512):
                                nc.vector.tensor_add(s1[:, ts(hf, 512)],
                                                     es[-2][:, ts(hf, 512)],
                                                     es[-1][:, ts(hf, 512)])
                        else:
                            nc.vector.tensor_add(s1, es[-2], es[-1])
                        s1s.append(s1)
                if jb % 4 == 3:
                    if jb == JT - 1 and not last:
                        pass      # deferred to finish_R
                    else:
                        s2 = s2_p.tile([128, WQ], F16, tag="s2",
                                       name=f"s2_{h}{qb}{jb}")
                        eng = (nc.gpsimd if len(s2s) < L2_POOL_COUNT and
                               not last else nc.vector)
                        if last and jb == JT - 1:
                            for hf in range(WQ // 512):
                                eng.tensor_add(s2[:, ts(hf, 512)],
                                               s1s[-2][:, ts(hf, 512)],
                                               s1s[-1][:, ts(hf, 512)])
                        else:
                            eng.tensor_add(s2, s1s[-2], s1s[-1])
                        s2s.append(s2)
                    if not last and jb == 7:
                        # L3 first half on Pool (plenty of time mid-block)
                        s3 = s34_p.tile([128, WQ], F16, tag="s34",
                                        name=f"s3_{h}{qb}{jb}")
                        nc.gpsimd.tensor_add(s3, s2s[-2], s2s[-1])
                        s3s.append(s3)
                if jb == JT - 1:
                    # AV accumulator evacuation frees the single PSUM slot.
                    # Emitted AFTER the tree adds so the R chain isn't stuck
                    # behind the 1.2us copy on DVE; the last block uses ACT
                    # (idle by then) to keep the DVE finish chain clean.
                    osb = osb_p.tile([128, WQ], F32, tag="osb",
                                     name=f"osb{h}{qb}")
                    if last:
                        nc.scalar.copy(out=osb, in_=o_acc)
                    else:
                        nc.vector.tensor_copy(osb, o_acc)
            rinv = rv_p.tile([128, WQ], F32, tag="rinv", name=f"rinv{h}{qb}")
            if last and R_pre:
                # only the final link (fresh s2_15) + recip after exp15
                R_ps = R_pre[0]
                for hf in range(WQ // 512):
                    nc.tensor.matmul(
                        out=R_ps[:, ts(hf, 512)], lhsT=ones_t,
                        rhs=s2s[3][:, ts(hf, 512)], start=False,
                        stop=True, skip_group_check=True)
                    nc.vector.reciprocal_approx_fast(
                        out=rinv[:, ts(hf, 512)], in_=R_ps[:, ts(hf, 512)])
            elif last:
                # R inline, per-half chain + reciprocal so the finish pipeline
                # (norm/pair/y/dma) starts on half 0 while half 1 reduces
                R_ps = ps_sc.tile([128, WQ], F32, tag="sc", name=f"R{h}{qb}")
                for hf in range(WQ // 512):
                    for u, s2 in enumerate(s2s):
                        nc.tensor.matmul(
                            out=R_ps[:, ts(hf, 512)], lhsT=ones_t,
                            rhs=s2[:, ts(hf, 512)], start=(u == 0),
                            stop=(u == len(s2s) - 1), skip_group_check=True)
                    nc.vector.reciprocal_approx_fast(
                        out=rinv[:, ts(hf, 512)], in_=R_ps[:, ts(hf, 512)])

            def finish_R():
                # deferred into the next block (FR_JB): the final tree column
                # (s1_15, s2_3, s3b) runs here too, far from the boundary
                s1t = s1_p.tile([128, WQ], F16, tag="s1", name=f"s1t{h}{qb}")
                nc.vector.tensor_add(s1t, es[-2], es[-1])
                s2t = s2_p.tile([128, WQ], F16, tag="s2", name=f"s2t{h}{qb}")
                nc.vector.tensor_add(s2t, s1s[-1], s1t)
                s3t = s34_p.tile([128, WQ], F16, tag="s34", name=f"s3t{h}{qb}")
                nc.vector.tensor_add(s3t, s2s[-1], s2t)
                R_ps = ps_sc.tile([128, WQ], F32, tag="sc", name=f"R{h}{qb}")
                for u, s3 in enumerate((s3s[0], s3t)):
                    for hf in range(WQ // 512):
                        nc.tensor.matmul(
                            out=R_ps[:, ts(hf, 512)], lhsT=ones_t,
                            rhs=s3[:, ts(hf, 512)], start=(u == 0),
                            stop=(u == 1), skip_group_check=True)
                nc.vector.reciprocal_approx_fast(out=rinv, in_=R_ps)

            def tail():
                onb = on_p.tile([128, WQ], F16, tag="onb", name=f"onb{h}{qb}")
                nc.gpsimd.tensor_mul(onb, osb, rinv)
                onb_t[(h, qb)] = onb
                if h == 1:
                    pr = y_p.tile([128, WQ], F16, tag="pair", name=f"pr1{qb}")
                    nc.gpsimd.tensor_add(pr, onb_t[(0, qb)], onb_t[(1, qb)])
                    pair_t[(0, qb)] = pr
                if h == 2:
                    # pre-sum heads 0..2 so the final y is one add per chunk
                    pr = y_p.tile([128, WQ], F16, tag="pair", name=f"pre3{qb}")
                    nc.gpsimd.tensor_add(pr, pair_t[(0, qb)], onb_t[(2, qb)])
                    pair_t[(2, qb)] = pr
                if h == 3:
                    emit_y(qb, 1)

            if last:
                # low-latency all-DVE finish, fully chunked per 512
                onb = on_p.tile([128, WQ], F16, tag="onb", name=f"onb{h}{qb}")
                onb_t[(h, qb)] = onb
                y_sb = y_p.tile([128, WQ], F16, tag="ysb", name=f"ysb{qb}")
                for hf in (TAIL_ORDER or range(WQ // 512)):
                    sl = slice(hf * 512, (hf + 1) * 512)
                    nc.vector.tensor_mul(onb[:, sl], osb[:, sl], rinv[:, sl])
                    nc.vector.tensor_add(y_sb[:, sl], pair_t[(2, qb)][:, sl],
                                         onb[:, sl])
                    # spread the final DMAs across queues so their DGE
                    # generations run in parallel, not serialized on one path
                    for qtr in range(TAIL_CHUNKS // 2):
                        c = hf * (TAIL_CHUNKS // 2) + qtr
                        w = WQ // TAIL_CHUNKS
                        csl = slice(c * w, (c + 1) * w)
                        eng = (nc.scalar, nc.gpsimd, nc.sync,
                               nc.vector)[TAILQS[c]]
                        eng.dma_start(out=yt_d[:, qb * WQ + c * w:
                                               qb * WQ + (c + 1) * w],
                                      in_=y_sb[:, csl])
                return None, None
            return finish_R, tail

        def emit_y(qb, chunks):
            y_sb = y_p.tile([128, WQ], F16, tag="ysb", name=f"ysb{qb}")
            for hf in range(chunks):
                w = WQ // chunks
                sl = slice(hf * w, (hf + 1) * w)
                nc.vector.tensor_add(y_sb[:, sl], pair_t[(2, qb)][:, sl],
                                     onb_t[(3, qb)][:, sl])
                (nc.sync if EMITY_SP else nc.gpsimd).dma_start(
                    out=yt_d[:, qb * WQ + hf * w: qb * WQ + (hf + 1) * w],
                    in_=y_sb[:, sl])

        def run_head(h, feed, pending, drain_feed, finish_prev):
            for qb in range(NQB):
                finish_prev, pending = attention_block(
                    h, qb, feed, pending, drain_feed, finish_prev)
                feed_gate[0] = None   # qb0 of head 0 feeds the prologue only
            return finish_prev, pending

        # prologue: run until head-0 (qb0) inputs exist
        pgen = prologue_steps()
        next(pgen)
        if DEBUG_DUMPS == 2:
            pass  # keep interleaved schedule; dumps added at end
        elif DEBUG_DUMPS:
            for _ in pgen:
                pass
            nc.sync.dma_start(out=dbg["xn"][:, :], in_=xn)
            nc.sync.dma_start(out=dbg["xnT"][:, :], in_=xnT)
            nc.sync.dma_start(out=dbg["qh0"][:, :], in_=qh_t[0])
            nc.sync.dma_start(out=dbg["kh0"][:, :], in_=kh_t[0])
            nc.sync.dma_start(out=dbg["vh0"][:, :], in_=vh_t[0])

        pending = None
        finishR = None
        feed_gate = [None]   # when set, only this generator may be pulled
        for h in range(HPC):
            gens = []
            if h == 0:
                gens.append(pgen)
                feed_gate[0] = pgen
            if h + 1 < HPC:
                gens.append(qkv_steps(h + 1))
            if h == 0 and DRAIN_FIRST:
                for _ in range(DRAIN_FIRST):
                    try:
                        next(gens[0])
                    except StopIteration:
                        break
            if DRAIN == 2 or (DRAIN == 1 and h == 0):
                while gens:
                    try:
                        next(gens[0])
                    except StopIteration:
                        gens.pop(0)

            def feed(gs=gens):
                while gs:
                    try:
                        next(gs[0])
                        return
                    except StopIteration:
                        gs.pop(0)

            def drain_feed(marker=None, gs=gens):
                while gs:
                    try:
                        st = next(gs[0])
                        if marker is not None and st == marker:
                            return
                    except StopIteration:
                        gs.pop(0)

            finishR, pending = run_head(h, feed, pending, drain_feed, finishR)
            while gens:
                try:
                    next(gens[0])
                except StopIteration:
                    gens.pop(0)
        if pending is not None:
            pending()
        if DEBUG_DUMPS == 2:
            nc.sync.dma_start(out=dbg["xnT"][:, 0:WQ], in_=onb_t[(1, 0)])
            nc.sync.dma_start(out=dbg["xn"][:, :], in_=qh_t[1])
            nc.sync.dma_start(out=dbg["xnT"][:, :], in_=kh_t[1])
            nc.sync.dma_start(out=dbg["qh0"][:, :], in_=vh_t[1])
            nc.sync.dma_start(out=dbg["kh0"][:, :], in_=qh_t[2])
            nc.sync.dma_start(out=dbg["vh0"][:, :], in_=vh_t[2])
            nc.sync.dma_start(out=dbg["e0"][:, :], in_=qh_t[3])
            nc.sync.dma_start(out=dbg["r0"][:, :], in_=vh_t[3])

    nc.finalize()
    return nc


def _make_runner(nc, n_cores=8):
    """Cached jitted multi-core executor (mirrors bass2jax.run_bass_via_pjrt,
    minus output-donation so it can be called repeatedly for timing)."""
    import jax
    import jax.numpy as jnp
    from jax.sharding import Mesh, PartitionSpec
    from jax.experimental.shard_map import shard_map
    from concourse import bass2jax, mybir as mb
    bass2jax.install_neuronx_cc_hook()

    partition_name = nc.partition_id_tensor.name if nc.partition_id_tensor else None
    in_names, out_names, out_avals, zero_outs = [], [], [], []
    for alloc in nc.m.functions[0].allocations:
        if not isinstance(alloc, mb.MemoryLocationSet):
            continue
        name = alloc.memorylocations[0].name
        if alloc.kind == "ExternalInput":
            if name != partition_name:
                in_names.append(name)
        elif alloc.kind == "ExternalOutput":
            out_names.append(name)
            shape = tuple(alloc.tensor_shape)
            dtype = mb.dt.np(alloc.dtype)
            out_avals.append(jax.core.ShapedArray(shape, dtype))
            zero_outs.append(np.zeros(shape, dtype))
    n_params = len(in_names)
    all_in_names = list(in_names) + list(out_names)
    if partition_name is not None:
        all_in_names.append(partition_name)

    def _body(*args):
        operands = list(args)
        if partition_name is not None:
            operands.append(bass2jax.partition_id_tensor())
        outs = bass2jax._bass_exec_p.bind(
            *operands,
            out_avals=tuple(out_avals),
            in_names=tuple(all_in_names),
            out_names=tuple(out_names),
            lowering_input_output_aliases=(),
            sim_require_finite=True,
            sim_require_nnan=True,
            nc=nc,
        )
        return tuple(outs)

    devices = jax.devices()[:n_cores]
    mesh = Mesh(np.asarray(devices), ("core",))
    in_specs = (PartitionSpec("core"),) * (n_params + len(out_names))
    out_specs = (PartitionSpec("core"),) * len(out_names)
    donate = tuple(range(n_params, n_params + len(out_names)))
    sharded = jax.jit(shard_map(_body, mesh=mesh, in_specs=in_specs,
                                out_specs=out_specs, check_rep=False),
                      donate_argnums=donate, keep_unused=True)

    def run(in_maps):
        concat_in = [np.concatenate([np.asarray(in_maps[c][k]) for c in range(n_cores)], axis=0)
                     for k in in_names]
        concat_zero = [np.concatenate([z] * n_cores, axis=0) for z in zero_outs]
        outs = sharded(*concat_in, *concat_zero)
        outs = [np.asarray(o) for o in outs]
        res = []
        for c in range(n_cores):
            d = {}
            for i, name in enumerate(out_names):
                per = outs[i].shape[0] // n_cores
                d[name] = outs[i][c * per:(c + 1) * per]
            res.append(d)
        return res, sharded, (in_names, zero_outs)

    return run


def _rope_tables():
    """cos/sin tables in [d, n] layout; token N-1 unrotated; sin sign-folded."""
    inv_freq = 1.0 / (10000.0 ** (np.arange(0, HEAD, 2, dtype=np.float64) / HEAD))
    pos = np.arange(N, dtype=np.float64)
    ang = pos[None, :] * np.repeat(inv_freq, 2)[:, None]        # [d, n]
    cos_t = np.cos(ang)
    sin_t = np.sin(ang)
    sign = np.where(np.arange(HEAD) % 2 == 0, -1.0, 1.0)[:, None]
    sin_t = sin_t * sign
    cos_t[:, N - 1] = 1.0
    sin_t[:, N - 1] = 0.0
    return cos_t.astype(F16_NP), sin_t.astype(F16_NP)


def _prep_core_inputs(x, ln_gamma, ln_beta, w_qkv, w_out):
    """Build the 8 per-core input maps (host-side layout/packing)."""
    cos_t, sin_t = _rope_tables()
    ident = np.eye(128, dtype=np.float32)
    swap = np.arange(HEAD) ^ 1

    # packed [cos_b0 | sin_b0 | cos_b1 | sin_b1]
    cossin = np.concatenate(
        [cos_t[:, 0:WQ], sin_t[:, 0:WQ], cos_t[:, WQ:N], sin_t[:, WQ:N]],
        axis=1).astype(F16_NP)
    identones = np.concatenate(
        [ident.astype(F16_NP), np.ones((128, 128), dtype=F16_NP)], axis=1)

    in_maps = []
    for c in range(8):
        b = c % 4
        g = c // 4
        wq_blocks = []
        for i in range(HPC):
            h = g * HPC + i
            Wq = w_qkv[h * HEAD:(h + 1) * HEAD, :] * ln_gamma[None, :]
            Wk = w_qkv[INNER + h * HEAD:INNER + (h + 1) * HEAD, :] * ln_gamma[None, :]
            Wv = w_qkv[2 * INNER + h * HEAD:2 * INNER + (h + 1) * HEAD, :] * ln_gamma[None, :]
            Wo = w_out[:, h * HEAD:(h + 1) * HEAD]
            Wvo = Wo @ Wv                                        # fold out-proj into V
            wq_blocks += [Wq.T, Wq[swap, :].T, Wk.T, Wk[swap, :].T, Wvo.T]
        wqkv_packed = np.concatenate(wq_blocks, axis=1)          # [128, W_COLS*128]
        # pre-tile x to [128, NT, 128]: [p, t, d] holds x[b, t*128+p, d]
        xb = np.ascontiguousarray(
            x[b].reshape(NT, 128, DIM).transpose(1, 0, 2),
            dtype=F16_NP)
        in_maps.append({
            "x": xb,
            "wqkv": wqkv_packed.astype(F16_NP),
            "cossin": cossin,
            "identones": identones,
        })
    return in_maps


def kernel(x, ln_gamma, ln_beta, w_qkv, w_out, b_out):
    x = np.asarray(x, dtype=np.float32)
    ln_gamma = np.asarray(ln_gamma, dtype=np.float32)
    ln_beta = np.asarray(ln_beta, dtype=np.float32)
    w_qkv = np.asarray(w_qkv, dtype=np.float32)
    w_out = np.asarray(w_out, dtype=np.float32)
    b_out = np.asarray(b_out, dtype=np.float32)
    assert np.allclose(ln_beta, 0.0), "beta folding not implemented"

    if "nc" not in _CACHE:
        _CACHE["nc"] = _build_nc()
    nc = _CACHE["nc"]

    in_maps = _prep_core_inputs(x, ln_gamma, ln_beta, w_qkv, w_out)
    _CACHE["last_in_maps"] = in_maps
    res = run_bass_kernel_spmd(nc, in_maps, list(range(8)))
    results = res.results

    out = np.empty((B, N, DIM), dtype=np.float32)
    for b in range(B):
        y0 = np.asarray(results[b]["yt"], dtype=np.float32)
        y1 = np.asarray(results[b + 4]["yt"], dtype=np.float32)
        out[b] = (y0 + y1).T + b_out[None, :]
    return out

